# revision 20
# baseline (speedup 1.0000x reference)
"""Trainium2 Bass kernel for nn_Decoder_17489106830107 (VMamba VSSBlock decoder).

Sharding: one (batch, scan-direction) pair per core (B=2 x K=4 = 8 cores).
The host pre-permutes each core's inputs into that core's scan coordinate
order (transpose / 180-rotation of the image), so all 8 cores run ONE
identical SPMD program for launch 1 (conv1x1 -> LN -> in_proj -> depthwise
conv -> x_proj/dt -> 16 hardware linear-recurrence scans on the DVE).
The host then scatter-adds the 4 directional outputs per batch and an
8-way token-parallel launch 2 does the merge epilogue (out_norm, gating,
out_proj, MLP).
"""
import numpy as np
from contextlib import ExitStack

import concourse.bacc as bacc
import concourse.bass as bass
import concourse.mybir as mybir
import concourse.tile as tile
from concourse.bass_utils import run_bass_kernel_spmd

F32 = mybir.dt.float32
AF = mybir.ActivationFunctionType
OP = mybir.AluOpType

B, C, H, W = 2, 256, 64, 64
D = 64
Di = 128
N = 16
R = 4
K = 4
L = H * W          # 4096
LC = 1024          # scan chunk
NCH = L // LC
EPS = 1e-5
T2 = 1024          # launch-2 token slice per core


# ---------------------------------------------------------------- host prep

def _perms():
    ar = np.arange(L)
    p1 = (ar % 64) * 64 + ar // 64
    return [ar, p1, ar[::-1].copy(), p1[::-1].copy()]


def _permute_kernel(w, k):
    if k == 0:
        return w
    if k == 1:
        return w.transpose(0, 2, 1)
    if k == 2:
        return w[:, ::-1, ::-1]
    return w.transpose(0, 2, 1)[:, ::-1, ::-1]


# ---------------------------------------------------------------- launch 1

def build_launch1():
    nc = bacc.Bacc("TRN2", target_bir_lowering=False, debug=False,
                   num_devices=8)

    def inp(name, shape):
        return nc.dram_tensor(name, shape, F32, kind="ExternalInput")

    xin = inp("xin", [3 * C, L])
    convT = inp("convT", [3 * C, D])
    conv_b = inp("conv_b", [D, 1])
    sel = inp("sel", [128, 2])
    ones1 = inp("ones1", [1, 128])
    Wp = inp("Wp", [D, 2 * Di])
    negq = inp("negq", [128, 2])
    bias_z = inp("bias_z", [128, 1])
    dwdiag = inp("dwdiag", [9, 128, 128])
    bias_dw = inp("bias_dw", [128, 1])
    xprojT = inp("xprojT", [Di, R + 2 * N])
    dtT = inp("dtT", [R, Di])
    dtb = inp("dtb", [Di, 1])
    A_in = inp("A_in", [Di, N])
    bsel = inp("bsel", [R + 2 * N, N * 128])
    csel = inp("csel", [R + 2 * N, N * 128])
    Ds_in = inp("Ds_in", [Di, 1])

    y_out = nc.dram_tensor("y_out", [Di, L], F32, kind="ExternalOutput")
    sz_out = nc.dram_tensor("sz_out", [Di, L], F32, kind="ExternalOutput")
    x_out = nc.dram_tensor("x_out", [D, L], F32, kind="ExternalOutput")

    with tile.TileContext(nc) as tc, ExitStack() as ctx:
        cpool = ctx.enter_context(tc.tile_pool(name="consts", bufs=1))
        main = ctx.enter_context(tc.tile_pool(name="main", bufs=1))

        # ---- const loads
        convT_sb = cpool.tile([128, 6, D], F32, tag="convT")
        nc.sync.dma_start(convT_sb[:], convT[:].rearrange("(c p) m -> p c m", p=128))
        conv_b_sb = cpool.tile([D, 1], F32, tag="convb")
        nc.sync.dma_start(conv_b_sb[:], conv_b[:])
        sel_sb = cpool.tile([128, 2], F32, tag="sel")
        nc.sync.dma_start(sel_sb[:], sel[:])
        ones1_sb = cpool.tile([1, 128], F32, tag="ones1")
        nc.sync.dma_start(ones1_sb[:], ones1[:])
        Wp_sb = cpool.tile([D, 2 * Di], F32, tag="Wp")
        nc.sync.dma_start(Wp_sb[:], Wp[:])
        negq_sb = cpool.tile([128, 2], F32, tag="negq")
        nc.sync.dma_start(negq_sb[:], negq[:])
        bias_z_sb = cpool.tile([128, 1], F32, tag="biasz")
        nc.sync.dma_start(bias_z_sb[:], bias_z[:])
        dwdiag_sb = cpool.tile([128, 9, 128], F32, tag="dwdiag")
        nc.sync.dma_start(dwdiag_sb[:], dwdiag[:].rearrange("t p f -> p t f"))
        bias_dw_sb = cpool.tile([128, 1], F32, tag="biasdw")
        nc.sync.dma_start(bias_dw_sb[:], bias_dw[:])
        xprojT_sb = cpool.tile([Di, R + 2 * N], F32, tag="xprojT")
        nc.sync.dma_start(xprojT_sb[:], xprojT[:])
        dtT_sb = cpool.tile([R, Di], F32, tag="dtT")
        nc.sync.dma_start(dtT_sb[:], dtT[:])
        dtb_sb = cpool.tile([Di, 1], F32, tag="dtb")
        nc.sync.dma_start(dtb_sb[:], dtb[:])
        A_sb = cpool.tile([Di, N], F32, tag="A")
        nc.sync.dma_start(A_sb[:], A_in[:])
        Ds_sb = cpool.tile([Di, 1], F32, tag="Ds")
        nc.sync.dma_start(Ds_sb[:], Ds_in[:])
        eps_sb = cpool.tile([128, 1], F32, tag="eps")
        nc.vector.memset(eps_sb[:], EPS)
        bsel_sb = cpool.tile([R + 2 * N, N * 128], F32, tag="bsel")
        nc.sync.dma_start(bsel_sb[:], bsel[:])
        csel_sb = cpool.tile([R + 2 * N, N * 128], F32, tag="csel")
        nc.sync.dma_start(csel_sb[:], csel[:])

        # ---- persistent activations
        xc_sb = main.tile([Di, L], F32, tag="xc")
        xdbl_sb = main.tile([R + 2 * N, L], F32, tag="xdbl")
        delta_sb = main.tile([Di, L], F32, tag="delta")
        du_sb = main.tile([Di, L], F32, tag="du")
        carry_sb = main.tile([Di, N], F32, tag="carry")

        with tc.tile_pool(name="imgp", bufs=1) as imgp:
            img = imgp.tile([Di, 66 * 66], F32, tag="img")
            nc.gpsimd.memset(img[:], 0.0)
            img3 = img[:].rearrange("p (h w) -> p h w", h=66)

            with tc.tile_pool(name="p1", bufs=1) as p1, \
                 tc.tile_pool(name="p1x", bufs=2) as p1x:
                lnin = p1.tile([128, L], F32, tag="lnin")

                # conv1x1: psum[f] accumulates over 6 channel chunks
                with tc.tile_pool(name="ps_conv", bufs=1,
                                  space="PSUM") as ps_conv:
                    cps = [ps_conv.tile([D, 512], F32, tag=f"cps{f}",
                                        name=f"cps{f}")
                           for f in range(8)]
                    for c in range(6):
                        xin_c = p1x.tile([128, L], F32, tag="xin")
                        nc.sync.dma_start(xin_c[:],
                                          xin[:][c * 128:(c + 1) * 128, :])
                        for f in range(8):
                            nc.tensor.matmul(cps[f][:], convT_sb[:, c, :],
                                             xin_c[:, f * 512:(f + 1) * 512],
                                             start=(c == 0), stop=(c == 5))
                    for f in range(8):
                        nc.scalar.activation(lnin[0:D, f * 512:(f + 1) * 512],
                                             cps[f][:], AF.Identity,
                                             bias=conv_b_sb[:, 0:1])
                nc.sync.dma_start(x_out[:], lnin[0:D, :])

                # ---- LN1 stats
                nc.scalar.square(lnin[D:2 * D, :], lnin[0:D, :])
                mu_b = p1.tile([128, L], F32, tag="mu_b")
                rs_b = p1.tile([128, L], F32, tag="rs_b")
                st0_sb = mu_b
                st1_sb = rs_b
                with tc.tile_pool(name="ps_st", bufs=1, space="PSUM") as ps_st:
                    for hh in range(2):
                        hsl = slice(hh * 2048, (hh + 1) * 2048)
                        st0 = ps_st.tile([1, L // 2], F32, tag="st0",
                                         name="st0")
                        st1 = ps_st.tile([1, L // 2], F32, tag="st1",
                                         name="st1")
                        for f in range(4):
                            fsl = slice(hh * 2048 + f * 512,
                                        hh * 2048 + (f + 1) * 512)
                            psl = slice(f * 512, (f + 1) * 512)
                            nc.tensor.matmul(st0[:, psl], sel_sb[:, 0:1],
                                             lnin[:, fsl], start=True,
                                             stop=True)
                            nc.tensor.matmul(st1[:, psl], sel_sb[:, 1:2],
                                             lnin[:, fsl], start=True,
                                             stop=True)
                        nc.scalar.copy(st0_sb[0:1, hsl], st0[:])
                        nc.scalar.copy(st1_sb[0:1, hsl], st1[:])

                s0r = p1.tile([128, 32], F32, tag="s0r")
                s1r = p1.tile([128, 32], F32, tag="s1r")
                nc.sync.dma_start(s0r[:], st0_sb[0:1, :])
                nc.sync.dma_start(s1r[:], st1_sb[0:1, :])
                m_r = p1.tile([128, 32], F32, tag="m_r")
                nc.scalar.mul(m_r[:], s0r[:], 1.0 / D)
                msq = p1.tile([128, 32], F32, tag="msq")
                nc.scalar.square(msq[:], m_r[:])
                v_r = p1.tile([128, 32], F32, tag="v_r")
                nc.vector.scalar_tensor_tensor(v_r[:], s1r[:], 1.0 / D, msq[:],
                                               OP.mult, OP.subtract)
                sd_r = p1.tile([128, 32], F32, tag="sd_r")
                nc.scalar.activation(sd_r[:], v_r[:], AF.Sqrt, bias=eps_sb[:, 0:1])
                rs_r = p1.tile([128, 32], F32, tag="rs_r")
                nc.vector.reciprocal(rs_r[:], sd_r[:])
                nc.sync.dma_start(mu_b[0:1, :], m_r[:])
                nc.sync.dma_start(rs_b[0:1, :], rs_r[:])
                nc.gpsimd.partition_broadcast(mu_b[:], mu_b[0:1, :])
                nc.gpsimd.partition_broadcast(rs_b[:], rs_b[0:1, :])

                # ---- in_proj + LN fixup
                with tc.tile_pool(name="ps_ip", bufs=4, space="PSUM") as ps_ip, \
                     tc.tile_pool(name="fix", bufs=3) as fix:
                    for mc in range(2):
                        for f in range(8):
                            fsl = slice(f * 512, (f + 1) * 512)
                            pp = ps_ip.tile([128, 512], F32, tag="ipps")
                            nc.tensor.matmul(pp[:],
                                             Wp_sb[:, mc * 128:(mc + 1) * 128],
                                             lnin[0:D, fsl],
                                             start=True, stop=True)
                            t1 = fix.tile([128, 512], F32, tag="t1")
                            nc.vector.scalar_tensor_tensor(
                                t1[:], mu_b[:, fsl], negq_sb[:, mc:mc + 1],
                                pp[:], OP.mult, OP.add)
                            if mc == 0:
                                r0 = f * 8
                                nc.vector.tensor_tensor(
                                    img3[:, 1 + r0:1 + r0 + 8, 1:65],
                                    t1[:].rearrange("p (r w) -> p r w", r=8),
                                    rs_b[:, fsl].rearrange("p (r w) -> p r w",
                                                           r=8),
                                    OP.mult)
                            else:
                                t2 = fix.tile([128, 512], F32, tag="t2")
                                nc.vector.tensor_tensor(t2[:], t1[:],
                                                        rs_b[:, fsl], OP.mult)
                                szt = fix.tile([128, 512], F32, tag="szt")
                                nc.scalar.activation(szt[:], t2[:],
                                                     AF.Silu,
                                                     bias=bias_z_sb[:, 0:1])
                                nc.sync.dma_start(sz_out[:][:, fsl], szt[:])

            # ---- depthwise conv 3x3 (diag matmuls, PSUM accumulation)
            with tc.tile_pool(name="ps_dw", bufs=2, space="PSUM") as ps_dw:
                for f in range(8):
                    r0 = f * 8
                    dps = ps_dw.tile([128, 512], F32, tag="dwps")
                    for t in range(9):
                        di_, dj = t // 3, t % 3
                        nc.tensor.matmul(
                            dps[:], dwdiag_sb[:, t, :],
                            img3[:, r0 + di_:r0 + di_ + 8, dj:dj + 64],
                            start=(t == 0), stop=(t == 8))
                    nc.scalar.activation(xc_sb[:, f * 512:(f + 1) * 512],
                                         dps[:], AF.Silu,
                                         bias=bias_dw_sb[:, 0:1])

        # ---- x_proj
        with tc.tile_pool(name="ps_xp", bufs=2, space="PSUM") as ps_xp:
            for f in range(8):
                fsl = slice(f * 512, (f + 1) * 512)
                xps = ps_xp.tile([R + 2 * N, 512], F32, tag="xpps")
                nc.tensor.matmul(xps[:], xprojT_sb[:], xc_sb[:, fsl],
                                 start=True, stop=True)
                nc.scalar.copy(xdbl_sb[:, fsl], xps[:])

        # ---- delta = softplus(dtT.T @ dts + dtb)
        with tc.tile_pool(name="ps_dt", bufs=2, space="PSUM") as ps_dt:
            for f in range(8):
                fsl = slice(f * 512, (f + 1) * 512)
                dps = ps_dt.tile([Di, 512], F32, tag="dtps")
                nc.tensor.matmul(dps[:], dtT_sb[:], xdbl_sb[0:R, fsl],
                                 start=True, stop=True)
                nc.scalar.activation(delta_sb[:, fsl], dps[:], AF.Sigmoid,
                                     bias=dtb_sb[:, 0:1], scale=-1.0)
        nc.scalar.activation(delta_sb[:], delta_sb[:], AF.Ln)
        nc.vector.scalar_tensor_tensor(du_sb[:], delta_sb[:], -1.0, xc_sb[:],
                                       OP.mult, OP.mult)

        # ---- the 16 scans, chunked over L
        with tc.tile_pool(name="ps_bb", bufs=2, space="PSUM") as ps_bb, \
             tc.tile_pool(name="nl", bufs=3) as nl, \
             tc.tile_pool(name="yp", bufs=2) as yp:
            for c in range(NCH):
                csl = slice(c * LC, (c + 1) * LC)
                y_acc = yp.tile([Di, LC], F32, tag="yacc")
                for n in range(N):
                    bb = ps_bb.tile([128, LC], F32, tag="bb")
                    for j in range(LC // 512):
                        nc.tensor.matmul(
                            bb[:, j * 512:(j + 1) * 512],
                            bsel_sb[:, n * 128:(n + 1) * 128],
                            xdbl_sb[:, c * LC + j * 512:c * LC + (j + 1) * 512],
                            start=True, stop=True)
                    cb = ps_bb.tile([128, LC], F32, tag="cb")
                    for j in range(LC // 512):
                        nc.tensor.matmul(
                            cb[:, j * 512:(j + 1) * 512],
                            csel_sb[:, n * 128:(n + 1) * 128],
                            xdbl_sb[:, c * LC + j * 512:c * LC + (j + 1) * 512],
                            start=True, stop=True)
                    da = nl.tile([Di, LC], F32, tag="da")
                    nc.scalar.activation(da[:], delta_sb[:, csl], AF.Exp,
                                         scale=A_sb[:, n:n + 1])
                    dbu = nl.tile([Di, LC], F32, tag="dbu")
                    nc.vector.tensor_tensor(dbu[:], du_sb[:, csl], bb[:],
                                            OP.mult)
                    h = nl.tile([Di, LC], F32, tag="h")
                    nc.vector.tensor_tensor_scan(
                        h[:], da[:], dbu[:],
                        0.0 if c == 0 else carry_sb[:, n:n + 1],
                        OP.mult, OP.add)
                    if c < NCH - 1:
                        nc.vector.tensor_copy(carry_sb[:, n:n + 1],
                                              h[:, LC - 1:LC])
                    if n == 0:
                        nc.vector.tensor_tensor(y_acc[:], h[:], cb[:], OP.mult)
                    else:
                        tmp = nl.tile([Di, LC], F32, tag="tmp")
                        nc.vector.tensor_tensor(tmp[:], h[:], cb[:], OP.mult)
                        nc.gpsimd.tensor_tensor(y_acc[:], y_acc[:], tmp[:],
                                                OP.add)
                y_f = yp.tile([Di, LC], F32, tag="yout")
                nc.vector.scalar_tensor_tensor(y_f[:], xc_sb[:, csl],
                                               Ds_sb[:, 0:1], y_acc[:],
                                               OP.mult, OP.add)
                nc.sync.dma_start(y_out[:][:, csl], y_f[:])

    nc.compile()
    return nc


# ---------------------------------------------------------------- launch 2

def build_launch2():
    nc = bacc.Bacc("TRN2", target_bir_lowering=False, debug=False,
                   num_devices=8)

    def inp(name, shape):
        return nc.dram_tensor(name, shape, F32, kind="ExternalInput")

    y_in = inp("y_in", [Di, T2])
    sz_in = inp("sz_in", [Di, T2])
    x_in = inp("x_in", [D, T2])
    ones128 = inp("ones128", [128, 1])
    onorm_g = inp("onorm_g", [Di, 1])
    onorm_b = inp("onorm_b", [Di, 1])
    oproj = inp("oproj", [Di, D])
    fc1p = inp("fc1p", [D, 2 * Di])
    bias1 = inp("bias1", [128, 2])
    fc2w = inp("fc2w", [2 * Di, D])
    fc2b = inp("fc2b", [D, 1])
    out = nc.dram_tensor("out", [D, T2], F32, kind="ExternalOutput")

    with tile.TileContext(nc) as tc, ExitStack() as ctx:
        po = ctx.enter_context(tc.tile_pool(name="main", bufs=1))
        ps = ctx.enter_context(tc.tile_pool(name="psum", bufs=1, space="PSUM"))

        y_sb = po.tile([Di, T2], F32, tag="y")
        nc.sync.dma_start(y_sb[:], y_in[:])
        sz_sb = po.tile([Di, T2], F32, tag="sz")
        nc.sync.dma_start(sz_sb[:], sz_in[:])
        x_sb = po.tile([D, T2], F32, tag="x")
        nc.sync.dma_start(x_sb[:], x_in[:])
        ones_sb = po.tile([128, 1], F32, tag="ones")
        nc.sync.dma_start(ones_sb[:], ones128[:])
        og_sb = po.tile([Di, 1], F32, tag="og")
        nc.sync.dma_start(og_sb[:], onorm_g[:])
        ob_sb = po.tile([Di, 1], F32, tag="ob")
        nc.sync.dma_start(ob_sb[:], onorm_b[:])
        op_sb = po.tile([Di, D], F32, tag="oproj")
        nc.sync.dma_start(op_sb[:], oproj[:])
        fc1_sb = po.tile([D, 2 * Di], F32, tag="fc1")
        nc.sync.dma_start(fc1_sb[:], fc1p[:])
        b1_sb = po.tile([128, 2], F32, tag="b1")
        nc.sync.dma_start(b1_sb[:], bias1[:])
        fc2_sb = po.tile([128, 2, D], F32, tag="fc2")
        nc.sync.dma_start(fc2_sb[:], fc2w[:].rearrange("(c p) m -> p c m", p=128))
        fc2b_sb = po.tile([D, 1], F32, tag="fc2b")
        nc.sync.dma_start(fc2b_sb[:], fc2b[:])
        eps_sb = po.tile([128, 1], F32, tag="eps")
        nc.vector.memset(eps_sb[:], EPS)

        def pln(src, parts, tag):
            """LayerNorm stats over the partition dim of src [parts, T2];
            returns broadcast (mu_b, rs_b) [parts, T2] tiles."""
            sq = po.tile([parts, T2], F32, tag=tag + "sq")
            nc.scalar.square(sq[:], src)
            st0_sb = po.tile([1, T2], F32, tag=tag + "st0sb")
            st1_sb = po.tile([1, T2], F32, tag=tag + "st1sb")
            with tc.tile_pool(name=tag + "ps_st", bufs=1,
                              space="PSUM") as ps_st:
                st0 = ps_st.tile([1, T2], F32, tag="st0")
                st1 = ps_st.tile([1, T2], F32, tag="st1")
                for f in range(T2 // 512):
                    fsl = slice(f * 512, (f + 1) * 512)
                    nc.tensor.matmul(st0[:, fsl], ones_sb[0:parts, :],
                                     src[:, fsl], start=True, stop=True)
                    nc.tensor.matmul(st1[:, fsl], ones_sb[0:parts, :],
                                     sq[:, fsl], start=True, stop=True)
                nc.scalar.copy(st0_sb[:], st0[:])
                nc.scalar.copy(st1_sb[:], st1[:])
            s0r = po.tile([128, T2 // 128], F32, tag=tag + "s0r")
            s1r = po.tile([128, T2 // 128], F32, tag=tag + "s1r")
            nc.sync.dma_start(s0r[:], st0_sb[:])
            nc.sync.dma_start(s1r[:], st1_sb[:])
            m_r = po.tile([128, T2 // 128], F32, tag=tag + "m")
            nc.scalar.mul(m_r[:], s0r[:], 1.0 / parts)
            msq = po.tile([128, T2 // 128], F32, tag=tag + "msq")
            nc.scalar.square(msq[:], m_r[:])
            v_r = po.tile([128, T2 // 128], F32, tag=tag + "v")
            nc.vector.scalar_tensor_tensor(v_r[:], s1r[:], 1.0 / parts,
                                           msq[:], OP.mult, OP.subtract)
            sd_r = po.tile([128, T2 // 128], F32, tag=tag + "sd")
            nc.scalar.activation(sd_r[:], v_r[:], AF.Sqrt, bias=eps_sb[:parts if False else 128, 0:1])
            rs_r = po.tile([128, T2 // 128], F32, tag=tag + "rs")
            nc.vector.reciprocal(rs_r[:], sd_r[:])
            mu1 = po.tile([1, T2], F32, tag=tag + "mu1")
            rs1 = po.tile([1, T2], F32, tag=tag + "rs1")
            nc.sync.dma_start(mu1[:], m_r[:])
            nc.sync.dma_start(rs1[:], rs_r[:])
            mu_b = po.tile([parts, T2], F32, tag=tag + "mub")
            rs_b = po.tile([parts, T2], F32, tag=tag + "rsb")
            nc.gpsimd.partition_broadcast(mu_b[:], mu1[:])
            nc.gpsimd.partition_broadcast(rs_b[:], rs1[:])
            return mu_b, rs_b

        # out_norm (over Di) + gate
        mu_b, rs_b = pln(y_sb[:], Di, "a")
        t1 = po.tile([Di, T2], F32, tag="t1")
        nc.vector.tensor_tensor(t1[:], y_sb[:], mu_b[:], OP.subtract)
        t2 = po.tile([Di, T2], F32, tag="t2")
        nc.vector.tensor_tensor(t2[:], t1[:], rs_b[:], OP.mult)
        t3 = po.tile([Di, T2], F32, tag="t3")
        nc.vector.tensor_scalar(t3[:], t2[:], og_sb[:, 0:1], ob_sb[:, 0:1],
                                OP.mult, OP.add)
        yg = po.tile([Di, T2], F32, tag="yg")
        nc.vector.tensor_tensor(yg[:], t3[:], sz_sb[:], OP.mult)

        # out_proj + residual ;  "mm" psum tag shared/serialized
        x2 = po.tile([D, T2], F32, tag="x2")
        opps = ps.tile([128, T2], F32, tag="mm")
        for f in range(T2 // 512):
            fsl = slice(f * 512, (f + 1) * 512)
            nc.tensor.matmul(opps[0:D, fsl], op_sb[:], yg[:, fsl],
                             start=True, stop=True)
        nc.vector.tensor_tensor(x2[:], opps[0:D, :], x_sb[:], OP.add)

        # LN2 (over D) -> fc1 -> gelu -> fc2 -> + residual
        mu2, rs2 = pln(x2[:], D, "b")
        h1 = po.tile([D, T2], F32, tag="h1")
        nc.vector.tensor_tensor(h1[:], x2[:], mu2[:], OP.subtract)
        hn = po.tile([D, T2], F32, tag="hn")
        nc.vector.tensor_tensor(hn[:], h1[:], rs2[:], OP.mult)

        g1 = po.tile([128, 2, T2], F32, tag="g1")
        for mc in range(2):
            fp = ps.tile([128, T2], F32, tag="mm")
            for f in range(T2 // 512):
                fsl = slice(f * 512, (f + 1) * 512)
                nc.tensor.matmul(fp[:, fsl],
                                 fc1_sb[:, mc * 128:(mc + 1) * 128],
                                 hn[:, fsl], start=True, stop=True)
            nc.scalar.activation(g1[:, mc, :], fp[:],
                                 AF.Gelu_apprx_tanh, bias=b1_sb[:, mc:mc + 1])
        f2 = ps.tile([128, T2], F32, tag="mm")
        for f in range(T2 // 512):
            fsl = slice(f * 512, (f + 1) * 512)
            for mc in range(2):
                nc.tensor.matmul(f2[0:D, fsl], fc2_sb[:, mc, :],
                                 g1[:, mc, fsl],
                                 start=(mc == 0), stop=(mc == 1))
        o_sb = po.tile([D, T2], F32, tag="o")
        nc.vector.scalar_tensor_tensor(o_sb[:], f2[0:D, :], fc2b_sb[:, 0:1],
                                       x2[:], OP.add, OP.add)
        nc.sync.dma_start(out[:], o_sb[:])

    nc.compile()
    return nc


# ---------------------------------------------------------------- host side

_CACHE = {}


def _get_programs():
    if "nc1" not in _CACHE:
        _CACHE["nc1"] = build_launch1()
        _CACHE["nc2"] = build_launch2()
    return _CACHE["nc1"], _CACHE["nc2"]


def _prep_inmaps(inputs):
    f32 = lambda a: np.ascontiguousarray(np.asarray(a), dtype=np.float32)
    conv_w = f32(inputs["conv_w"])
    conv_b = f32(inputs["conv_b"])
    ln1_g, ln1_b = f32(inputs["ln1_g"]), f32(inputs["ln1_b"])
    in_proj_w = f32(inputs["in_proj_w"])
    dw_w_all = f32(inputs["conv_dw_w"])[:, 0]
    dw_b = f32(inputs["conv_dw_b"])
    x_proj_w = f32(inputs["x_proj_w"])
    dt_proj_w = f32(inputs["dt_proj_w"])
    dt_proj_b = f32(inputs["dt_proj_b"])
    A = np.exp(f32(inputs["A_logs"])).reshape(K, Di, N).astype(np.float32)
    Ds = f32(inputs["Ds"]).reshape(K, Di)

    Wp = (ln1_g[:, None] * in_proj_w).astype(np.float32)        # [64, 256]
    q = Wp.sum(0)
    bias_full = (ln1_b @ in_proj_w).astype(np.float32)          # [256]
    negq = np.ascontiguousarray(np.stack([-q[:Di], -q[Di:]], 1), np.float32)
    sel = np.zeros((128, 2), np.float32)
    sel[:D, 0] = 1.0
    sel[D:, 1] = 1.0
    ones1 = np.ones((1, 128), np.float32)

    Ps = _perms()
    x123 = [np.concatenate([f32(inputs["x1"])[b], f32(inputs["x2"])[b],
                            f32(inputs["x3"])[b]], 0).reshape(3 * C, L)
            for b in range(B)]

    bsel_np = np.zeros((R + 2 * N, N * 128), np.float32)
    csel_np = np.zeros((R + 2 * N, N * 128), np.float32)
    for n in range(N):
        bsel_np[R + n, n * 128:(n + 1) * 128] = 1.0
        csel_np[R + N + n, n * 128:(n + 1) * 128] = 1.0
    shared = {
        "bsel": bsel_np, "csel": csel_np,
        "convT": np.ascontiguousarray(conv_w.T),
        "conv_b": conv_b.reshape(D, 1).copy(),
        "sel": sel, "ones1": ones1, "Wp": Wp, "negq": negq,
        "bias_z": bias_full[Di:].reshape(Di, 1).copy(),
    }
    in_maps = []
    for core in range(8):
        b, k = core // 4, core % 4
        dw_w = _permute_kernel(dw_w_all, k)
        wsum = dw_w.sum((1, 2))
        dwdiag = np.zeros((9, 128, 128), np.float32)
        for t in range(9):
            np.fill_diagonal(dwdiag[t], dw_w[:, t // 3, t % 3])
        in_maps.append({
            **shared,
            "xin": np.ascontiguousarray(x123[b][:, Ps[k]]),
            "dwdiag": dwdiag,
            "bias_dw": (dw_b + bias_full[:Di] * wsum).reshape(Di, 1)
                        .astype(np.float32),
            "xprojT": np.ascontiguousarray(x_proj_w[k].T),
            "dtT": np.ascontiguousarray(dt_proj_w[k].T),
            "dtb": (-dt_proj_b[k]).reshape(Di, 1).copy(),
            "A_in": np.ascontiguousarray(A[k]),
            "Ds_in": Ds[k].reshape(Di, 1).copy(),
        })
    return in_maps, Ps


def _prep_inmaps2(inputs, y_merged, sz_full, x_full):
    f32 = lambda a: np.ascontiguousarray(np.asarray(a), dtype=np.float32)
    ln2_g, ln2_b = f32(inputs["ln2_g"]), f32(inputs["ln2_b"])
    fc1_w, fc1_b = f32(inputs["fc1_w"]), f32(inputs["fc1_b"])
    fc1p = (ln2_g[:, None] * fc1_w).astype(np.float32)
    bias1 = (ln2_b @ fc1_w + fc1_b).astype(np.float32)
    shared = {
        "ones128": np.ones((128, 1), np.float32),
        "onorm_g": f32(inputs["out_norm_g"]).reshape(Di, 1).copy(),
        "onorm_b": f32(inputs["out_norm_b"]).reshape(Di, 1).copy(),
        "oproj": f32(inputs["out_proj_w"]),
        "fc1p": fc1p,
        "bias1": np.ascontiguousarray(np.stack([bias1[:128], bias1[128:]], 1),
                                      np.float32),
        "fc2w": f32(inputs["fc2_w"]),
        "fc2b": f32(inputs["fc2_b"]).reshape(D, 1).copy(),
    }
    in_maps = []
    for core in range(8):
        b, sl = core // 4, slice((core % 4) * T2, (core % 4 + 1) * T2)
        in_maps.append({
            **shared,
            "y_in": np.ascontiguousarray(y_merged[b][:, sl]),
            "sz_in": np.ascontiguousarray(sz_full[b][:, sl]),
            "x_in": np.ascontiguousarray(x_full[b][:, sl]),
        })
    return in_maps


def kernel(**inputs):
    nc1, nc2 = _get_programs()
    in_maps, Ps = _prep_inmaps(inputs)
    res1 = run_bass_kernel_spmd(nc1, in_maps, list(range(8))).results

    y_merged = np.zeros((B, Di, L), np.float32)
    sz_full = [None] * B
    x_full = [None] * B
    for core in range(8):
        b, k = core // 4, core % 4
        y_merged[b][:, Ps[k]] += res1[core]["y_out"]
        if k == 0:
            sz_full[b] = res1[core]["sz_out"]
            x_full[b] = res1[core]["x_out"]

    in_maps2 = _prep_inmaps2(inputs, y_merged, sz_full, x_full)
    res2 = run_bass_kernel_spmd(nc2, in_maps2, list(range(8))).results

    out = np.zeros((B, D, L), np.float32)
    for core in range(8):
        b, sl = core // 4, slice((core % 4) * T2, (core % 4 + 1) * T2)
        out[b][:, sl] = res2[core]["out"]
    return out.reshape(B, D, H, W)


# revision 24
# speedup vs baseline: 1.0724x; 1.0724x over previous
"""Trainium2 Bass kernel for nn_Decoder_17489106830107 (VMamba VSSBlock decoder).

Sharding: one (batch, scan-direction) pair per core (B=2 x K=4 = 8 cores).
The host pre-permutes each core's inputs into that core's scan coordinate
order (transpose / 180-rotation of the image), so all 8 cores run ONE
identical SPMD program for launch 1 (conv1x1 -> LN -> in_proj -> depthwise
conv -> x_proj/dt -> 16 hardware linear-recurrence scans on the DVE).
The host then scatter-adds the 4 directional outputs per batch and an
8-way token-parallel launch 2 does the merge epilogue (out_norm, gating,
out_proj, MLP).
"""
import numpy as np
from contextlib import ExitStack

import concourse.bacc as bacc
import concourse.bass as bass
import concourse.mybir as mybir
import concourse.tile as tile
from concourse.bass_utils import run_bass_kernel_spmd
import ml_dtypes

F32 = mybir.dt.float32
F32R = mybir.dt.float32r
BF16 = mybir.dt.bfloat16
AF = mybir.ActivationFunctionType
OP = mybir.AluOpType

B, C, H, W = 2, 256, 64, 64
D = 64
Di = 128
N = 16
R = 4
K = 4
L = H * W          # 4096
LC = 1024          # scan chunk
NCH = L // LC
EPS = 1e-5
T2 = 1024          # launch-2 token slice per core


# ---------------------------------------------------------------- host prep

def _perms():
    ar = np.arange(L)
    p1 = (ar % 64) * 64 + ar // 64
    return [ar, p1, ar[::-1].copy(), p1[::-1].copy()]


def _permute_kernel(w, k):
    if k == 0:
        return w
    if k == 1:
        return w.transpose(0, 2, 1)
    if k == 2:
        return w[:, ::-1, ::-1]
    return w.transpose(0, 2, 1)[:, ::-1, ::-1]


# ---------------------------------------------------------------- launch 1

def _r(ap):
    return ap.bitcast(F32R)


def build_launch1():
    nc = bacc.Bacc("TRN2", target_bir_lowering=False, debug=False,
                   num_devices=8)

    def inp(name, shape):
        return nc.dram_tensor(name, shape, F32, kind="ExternalInput")

    xin = inp("xin", [3 * C, L])
    convT = inp("convT", [3 * C, D])
    conv_b = inp("conv_b", [D, 1])
    sel = inp("sel", [128, 2])
    ones1 = inp("ones1", [1, 128])
    Wp = nc.dram_tensor("Wp", [D, 2 * Di], BF16,
                        kind="ExternalInput")
    negq = inp("negq", [128, 2])
    bias_z = inp("bias_z", [128, 1])
    dwdiag = nc.dram_tensor("dwdiag", [9, 128, 128], BF16,
                            kind="ExternalInput")
    bias_dw = inp("bias_dw", [128, 1])
    xprojT = nc.dram_tensor("xprojT", [Di, R + 2 * N], BF16,
                            kind="ExternalInput")
    dtT = nc.dram_tensor("dtT", [R, Di], BF16,
                         kind="ExternalInput")
    dtb = inp("dtb", [Di, 1])
    A_in = inp("A_in", [Di, N])
    bsel = nc.dram_tensor("bsel", [R + 2 * N, N * 128], BF16,
                          kind="ExternalInput")
    csel = nc.dram_tensor("csel", [R + 2 * N, N * 128], BF16,
                          kind="ExternalInput")
    Ds_in = inp("Ds_in", [Di, 1])

    y_out = nc.dram_tensor("y_out", [Di, L], F32, kind="ExternalOutput")
    sz_out = nc.dram_tensor("sz_out", [Di, L], F32, kind="ExternalOutput")
    x_out = nc.dram_tensor("x_out", [D, L], F32, kind="ExternalOutput")

    with tile.TileContext(nc) as tc, ExitStack() as ctx:
        cpool = ctx.enter_context(tc.tile_pool(name="consts", bufs=1))
        main = ctx.enter_context(tc.tile_pool(name="main", bufs=1))

        # ---- const loads
        convT_sb = cpool.tile([128, 6, D], F32, tag="convT")
        nc.sync.dma_start(convT_sb[:], convT[:].rearrange("(c p) m -> p c m", p=128))
        conv_b_sb = cpool.tile([D, 1], F32, tag="convb")
        nc.sync.dma_start(conv_b_sb[:], conv_b[:])
        sel_sb = cpool.tile([128, 2], F32, tag="sel")
        nc.sync.dma_start(sel_sb[:], sel[:])
        ones1_sb = cpool.tile([1, 128], F32, tag="ones1")
        nc.sync.dma_start(ones1_sb[:], ones1[:])
        Wp_sb = cpool.tile([D, 2 * Di], BF16, tag="Wp")
        nc.sync.dma_start(Wp_sb[:], Wp[:])
        negq_sb = cpool.tile([128, 2], F32, tag="negq")
        nc.sync.dma_start(negq_sb[:], negq[:])
        bias_z_sb = cpool.tile([128, 1], F32, tag="biasz")
        nc.sync.dma_start(bias_z_sb[:], bias_z[:])
        dwdiag_sb = cpool.tile([128, 9, 128], BF16, tag="dwdiag")
        nc.sync.dma_start(dwdiag_sb[:], dwdiag[:].rearrange("t p f -> p t f"))
        bias_dw_sb = cpool.tile([128, 1], F32, tag="biasdw")
        nc.sync.dma_start(bias_dw_sb[:], bias_dw[:])
        xprojT_sb = cpool.tile([Di, R + 2 * N], BF16, tag="xprojT")
        nc.sync.dma_start(xprojT_sb[:], xprojT[:])
        dtT_sb = cpool.tile([R, Di], BF16, tag="dtT")
        nc.sync.dma_start(dtT_sb[:], dtT[:])
        dtb_sb = cpool.tile([Di, 1], F32, tag="dtb")
        nc.sync.dma_start(dtb_sb[:], dtb[:])
        A_sb = cpool.tile([Di, N], F32, tag="A")
        nc.sync.dma_start(A_sb[:], A_in[:])
        Ds_sb = cpool.tile([Di, 1], F32, tag="Ds")
        nc.sync.dma_start(Ds_sb[:], Ds_in[:])
        eps_sb = cpool.tile([128, 1], F32, tag="eps")
        nc.vector.memset(eps_sb[:], EPS)
        bsel_sb = cpool.tile([R + 2 * N, N * 128], BF16, tag="bsel")
        nc.sync.dma_start(bsel_sb[:], bsel[:])
        csel_sb = cpool.tile([R + 2 * N, N * 128], BF16, tag="csel")
        nc.sync.dma_start(csel_sb[:], csel[:])

        # ---- persistent activations
        xc_sb = main.tile([Di, L], BF16, tag="xc")
        xdbl_bf = main.tile([R + 2 * N, L], BF16, tag="xdblbf")
        delta_sb = main.tile([Di, L], F32, tag="delta")
        du_sb = main.tile([Di, L], F32, tag="du")
        carry_sb = main.tile([Di, N], F32, tag="carry")

        with tc.tile_pool(name="imgp", bufs=1) as imgp:
            img = imgp.tile([Di, 66 * 66], BF16, tag="img")
            nc.gpsimd.memset(img[:], 0.0)
            img3 = img[:].rearrange("p (h w) -> p h w", h=66)

            with tc.tile_pool(name="p1", bufs=1) as p1, \
                 tc.tile_pool(name="p1x", bufs=2) as p1x:
                lnin = p1.tile([128, L], F32, tag="lnin")

                # conv1x1: psum[f] accumulates over 6 channel chunks
                with tc.tile_pool(name="ps_conv", bufs=1,
                                  space="PSUM") as ps_conv:
                    cps = [ps_conv.tile([D, 512], F32, tag=f"cps{f}",
                                        name=f"cps{f}")
                           for f in range(8)]
                    for c in range(6):
                        xin_c = p1x.tile([128, L], F32, tag="xin")
                        nc.sync.dma_start(xin_c[:],
                                          xin[:][c * 128:(c + 1) * 128, :])
                        for f in range(8):
                            nc.tensor.matmul(cps[f][:], convT_sb[:, c, :],
                                             xin_c[:, f * 512:(f + 1) * 512],
                                             start=(c == 0), stop=(c == 5))
                    for f in range(8):
                        nc.scalar.activation(lnin[0:D, f * 512:(f + 1) * 512],
                                             cps[f][:], AF.Identity,
                                             bias=conv_b_sb[:, 0:1])
                nc.sync.dma_start(x_out[:], lnin[0:D, :])
                lnin_bf = p1.tile([D, L], BF16, tag="lninbf")
                nc.scalar.copy(lnin_bf[:], lnin[0:D, :])

                # ---- LN1 stats
                nc.scalar.square(lnin[D:2 * D, :], lnin[0:D, :])
                mu_b = p1.tile([128, L], F32, tag="mu_b")
                rs_b = p1.tile([128, L], F32, tag="rs_b")
                st0_sb = mu_b
                st1_sb = rs_b
                with tc.tile_pool(name="ps_st", bufs=1, space="PSUM") as ps_st:
                    for hh in range(2):
                        hsl = slice(hh * 2048, (hh + 1) * 2048)
                        st0 = ps_st.tile([1, L // 2], F32, tag="st0",
                                         name="st0")
                        st1 = ps_st.tile([1, L // 2], F32, tag="st1",
                                         name="st1")
                        for f in range(4):
                            fsl = slice(hh * 2048 + f * 512,
                                        hh * 2048 + (f + 1) * 512)
                            psl = slice(f * 512, (f + 1) * 512)
                            nc.tensor.matmul(st0[:, psl], sel_sb[:, 0:1],
                                             lnin[:, fsl], start=True,
                                             stop=True)
                            nc.tensor.matmul(st1[:, psl], sel_sb[:, 1:2],
                                             lnin[:, fsl], start=True,
                                             stop=True)
                        nc.scalar.copy(st0_sb[0:1, hsl], st0[:])
                        nc.scalar.copy(st1_sb[0:1, hsl], st1[:])

                s0r = p1.tile([128, 32], F32, tag="s0r")
                s1r = p1.tile([128, 32], F32, tag="s1r")
                nc.sync.dma_start(s0r[:], st0_sb[0:1, :])
                nc.sync.dma_start(s1r[:], st1_sb[0:1, :])
                m_r = p1.tile([128, 32], F32, tag="m_r")
                nc.scalar.mul(m_r[:], s0r[:], 1.0 / D)
                msq = p1.tile([128, 32], F32, tag="msq")
                nc.scalar.square(msq[:], m_r[:])
                v_r = p1.tile([128, 32], F32, tag="v_r")
                nc.vector.scalar_tensor_tensor(v_r[:], s1r[:], 1.0 / D, msq[:],
                                               OP.mult, OP.subtract)
                sd_r = p1.tile([128, 32], F32, tag="sd_r")
                nc.scalar.activation(sd_r[:], v_r[:], AF.Sqrt, bias=eps_sb[:, 0:1])
                rs_r = p1.tile([128, 32], F32, tag="rs_r")
                nc.vector.reciprocal(rs_r[:], sd_r[:])
                nc.sync.dma_start(mu_b[0:1, :], m_r[:])
                nc.sync.dma_start(rs_b[0:1, :], rs_r[:])
                nc.gpsimd.partition_broadcast(mu_b[:], mu_b[0:1, :])
                nc.gpsimd.partition_broadcast(rs_b[:], rs_b[0:1, :])

                # ---- in_proj + LN fixup
                with tc.tile_pool(name="ps_ip", bufs=4, space="PSUM") as ps_ip, \
                     tc.tile_pool(name="fix", bufs=3) as fix:
                    for mc in range(2):
                        for f in range(8):
                            fsl = slice(f * 512, (f + 1) * 512)
                            pp = ps_ip.tile([128, 512], F32, tag="ipps")
                            nc.tensor.matmul(pp[:],
                                             Wp_sb[:, mc * 128:(mc + 1) * 128],
                                             lnin_bf[:, fsl],
                                             start=True, stop=True)
                            t1 = fix.tile([128, 512], F32, tag="t1")
                            nc.vector.scalar_tensor_tensor(
                                t1[:], mu_b[:, fsl], negq_sb[:, mc:mc + 1],
                                pp[:], OP.mult, OP.add)
                            if mc == 0:
                                r0 = f * 8
                                nc.vector.tensor_tensor(
                                    img3[:, 1 + r0:1 + r0 + 8, 1:65],
                                    t1[:].rearrange("p (r w) -> p r w", r=8),
                                    rs_b[:, fsl].rearrange("p (r w) -> p r w",
                                                           r=8),
                                    OP.mult)
                            else:
                                t2 = fix.tile([128, 512], F32, tag="t2")
                                nc.vector.tensor_tensor(t2[:], t1[:],
                                                        rs_b[:, fsl], OP.mult)
                                szt = fix.tile([128, 512], F32, tag="szt")
                                nc.scalar.activation(szt[:], t2[:],
                                                     AF.Silu,
                                                     bias=bias_z_sb[:, 0:1])
                                nc.sync.dma_start(sz_out[:][:, fsl], szt[:])

            # ---- depthwise conv 3x3 (diag matmuls, PSUM accumulation)
            with tc.tile_pool(name="ps_dw", bufs=2, space="PSUM") as ps_dw:
                for f in range(8):
                    r0 = f * 8
                    dps = ps_dw.tile([128, 512], F32, tag="dwps")
                    for t in range(9):
                        di_, dj = t // 3, t % 3
                        nc.tensor.matmul(
                            dps[:], dwdiag_sb[:, t, :],
                            img3[:, r0 + di_:r0 + di_ + 8, dj:dj + 64],
                            start=(t == 0), stop=(t == 8))
                    nc.scalar.activation(xc_sb[:, f * 512:(f + 1) * 512],
                                         dps[:], AF.Silu,
                                         bias=bias_dw_sb[:, 0:1])

        # ---- x_proj (evac to bf16 for broadcasts + fp32 dts rows)
        dtp = ctx.enter_context(tc.tile_pool(name="dtp", bufs=1))
        dts_sb = dtp.tile([R, L], BF16, tag="dts")
        with tc.tile_pool(name="ps_xp", bufs=2, space="PSUM") as ps_xp:
            for f in range(8):
                fsl = slice(f * 512, (f + 1) * 512)
                xps = ps_xp.tile([R + 2 * N, 512], F32, tag="xpps")
                nc.tensor.matmul(xps[:], xprojT_sb[:], xc_sb[:, fsl],
                                 start=True, stop=True)
                nc.scalar.copy(xdbl_bf[:, fsl], xps[:])
                nc.scalar.copy(dts_sb[:, fsl], xps[0:R, :])

        # ---- delta = softplus(dtT.T @ dts + dtb)
        with tc.tile_pool(name="ps_dt", bufs=2, space="PSUM") as ps_dt:
            for f in range(8):
                fsl = slice(f * 512, (f + 1) * 512)
                dps = ps_dt.tile([Di, 512], F32, tag="dtps")
                nc.tensor.matmul(dps[:], dtT_sb[:], dts_sb[:, fsl],
                                 start=True, stop=True)
                nc.scalar.activation(delta_sb[:, fsl], dps[:], AF.Sigmoid,
                                     bias=dtb_sb[:, 0:1], scale=-1.0)
        nc.scalar.activation(delta_sb[:], delta_sb[:], AF.Ln)
        nc.vector.scalar_tensor_tensor(du_sb[:], delta_sb[:], -1.0, xc_sb[:],
                                       OP.mult, OP.mult)

        # ---- the 16 scans, chunked over L
        import os as _os
        if _os.environ.get("SKIP_NLOOP"):
            return _finish(nc)
        with tc.tile_pool(name="ps_bb", bufs=2, space="PSUM") as ps_bb, \
             tc.tile_pool(name="nl", bufs=3) as nl, \
             tc.tile_pool(name="yp", bufs=2) as yp:
            for c in range(NCH):
                csl = slice(c * LC, (c + 1) * LC)
                y_acc = yp.tile([Di, LC], F32, tag="yacc")
                for n in range(N):
                    bb = ps_bb.tile([128, LC], F32, tag="bb")
                    for j in range(LC // 512):
                        nc.tensor.matmul(
                            bb[:, j * 512:(j + 1) * 512],
                            bsel_sb[:, n * 128:(n + 1) * 128],
                            xdbl_bf[:, c * LC + j * 512:c * LC + (j + 1) * 512],
                            start=True, stop=True)
                    cb = ps_bb.tile([128, LC], F32, tag="cb")
                    for j in range(LC // 512):
                        nc.tensor.matmul(
                            cb[:, j * 512:(j + 1) * 512],
                            csel_sb[:, n * 128:(n + 1) * 128],
                            xdbl_bf[:, c * LC + j * 512:c * LC + (j + 1) * 512],
                            start=True, stop=True)
                    da = nl.tile([Di, LC], F32, tag="da")
                    nc.scalar.activation(da[:], delta_sb[:, csl], AF.Exp,
                                         scale=A_sb[:, n:n + 1])
                    dbu = nl.tile([Di, LC], F32, tag="dbu")
                    nc.vector.tensor_tensor(dbu[:], du_sb[:, csl], bb[:],
                                            OP.mult)
                    h = nl.tile([Di, LC], F32, tag="h")
                    nc.vector.tensor_tensor_scan(
                        h[:], da[:], dbu[:],
                        0.0 if c == 0 else carry_sb[:, n:n + 1],
                        OP.mult, OP.add)
                    if c < NCH - 1:
                        nc.vector.tensor_copy(carry_sb[:, n:n + 1],
                                              h[:, LC - 1:LC])
                    if n == 0:
                        nc.vector.tensor_tensor(y_acc[:], h[:], cb[:], OP.mult)
                    else:
                        tmp = nl.tile([Di, LC], F32, tag="tmp")
                        nc.vector.tensor_tensor(tmp[:], h[:], cb[:], OP.mult)
                        nc.gpsimd.tensor_tensor(y_acc[:], y_acc[:], tmp[:],
                                                OP.add)
                y_f = yp.tile([Di, LC], F32, tag="yout")
                nc.vector.scalar_tensor_tensor(y_f[:], xc_sb[:, csl],
                                               Ds_sb[:, 0:1], y_acc[:],
                                               OP.mult, OP.add)
                nc.sync.dma_start(y_out[:][:, csl], y_f[:])

    nc.compile()
    return nc


def _finish(nc):
    return nc


# ---------------------------------------------------------------- launch 2

def build_launch2():
    nc = bacc.Bacc("TRN2", target_bir_lowering=False, debug=False,
                   num_devices=8)

    def inp(name, shape):
        return nc.dram_tensor(name, shape, F32, kind="ExternalInput")

    y_in = inp("y_in", [Di, T2])
    sz_in = inp("sz_in", [Di, T2])
    x_in = inp("x_in", [D, T2])
    ones128 = inp("ones128", [128, 1])
    onorm_g = inp("onorm_g", [Di, 1])
    onorm_b = inp("onorm_b", [Di, 1])
    oproj = inp("oproj", [Di, D])
    fc1p = inp("fc1p", [D, 2 * Di])
    bias1 = inp("bias1", [128, 2])
    fc2w = inp("fc2w", [2 * Di, D])
    fc2b = inp("fc2b", [D, 1])
    out = nc.dram_tensor("out", [D, T2], F32, kind="ExternalOutput")

    with tile.TileContext(nc) as tc, ExitStack() as ctx:
        po = ctx.enter_context(tc.tile_pool(name="main", bufs=1))
        ps = ctx.enter_context(tc.tile_pool(name="psum", bufs=1, space="PSUM"))

        y_sb = po.tile([Di, T2], F32, tag="y")
        nc.sync.dma_start(y_sb[:], y_in[:])
        sz_sb = po.tile([Di, T2], F32, tag="sz")
        nc.sync.dma_start(sz_sb[:], sz_in[:])
        x_sb = po.tile([D, T2], F32, tag="x")
        nc.sync.dma_start(x_sb[:], x_in[:])
        ones_sb = po.tile([128, 1], F32, tag="ones")
        nc.sync.dma_start(ones_sb[:], ones128[:])
        og_sb = po.tile([Di, 1], F32, tag="og")
        nc.sync.dma_start(og_sb[:], onorm_g[:])
        ob_sb = po.tile([Di, 1], F32, tag="ob")
        nc.sync.dma_start(ob_sb[:], onorm_b[:])
        op_sb = po.tile([Di, D], F32, tag="oproj")
        nc.sync.dma_start(op_sb[:], oproj[:])
        fc1_sb = po.tile([D, 2 * Di], F32, tag="fc1")
        nc.sync.dma_start(fc1_sb[:], fc1p[:])
        b1_sb = po.tile([128, 2], F32, tag="b1")
        nc.sync.dma_start(b1_sb[:], bias1[:])
        fc2_sb = po.tile([128, 2, D], F32, tag="fc2")
        nc.sync.dma_start(fc2_sb[:], fc2w[:].rearrange("(c p) m -> p c m", p=128))
        fc2b_sb = po.tile([D, 1], F32, tag="fc2b")
        nc.sync.dma_start(fc2b_sb[:], fc2b[:])
        eps_sb = po.tile([128, 1], F32, tag="eps")
        nc.vector.memset(eps_sb[:], EPS)

        def pln(src, parts, tag):
            """LayerNorm stats over the partition dim of src [parts, T2];
            returns broadcast (mu_b, rs_b) [parts, T2] tiles."""
            sq = po.tile([parts, T2], F32, tag=tag + "sq")
            nc.scalar.square(sq[:], src)
            st0_sb = po.tile([1, T2], F32, tag=tag + "st0sb")
            st1_sb = po.tile([1, T2], F32, tag=tag + "st1sb")
            with tc.tile_pool(name=tag + "ps_st", bufs=1,
                              space="PSUM") as ps_st:
                st0 = ps_st.tile([1, T2], F32, tag="st0")
                st1 = ps_st.tile([1, T2], F32, tag="st1")
                for f in range(T2 // 512):
                    fsl = slice(f * 512, (f + 1) * 512)
                    nc.tensor.matmul(st0[:, fsl], ones_sb[0:parts, :],
                                     src[:, fsl], start=True, stop=True)
                    nc.tensor.matmul(st1[:, fsl], ones_sb[0:parts, :],
                                     sq[:, fsl], start=True, stop=True)
                nc.scalar.copy(st0_sb[:], st0[:])
                nc.scalar.copy(st1_sb[:], st1[:])
            s0r = po.tile([128, T2 // 128], F32, tag=tag + "s0r")
            s1r = po.tile([128, T2 // 128], F32, tag=tag + "s1r")
            nc.sync.dma_start(s0r[:], st0_sb[:])
            nc.sync.dma_start(s1r[:], st1_sb[:])
            m_r = po.tile([128, T2 // 128], F32, tag=tag + "m")
            nc.scalar.mul(m_r[:], s0r[:], 1.0 / parts)
            msq = po.tile([128, T2 // 128], F32, tag=tag + "msq")
            nc.scalar.square(msq[:], m_r[:])
            v_r = po.tile([128, T2 // 128], F32, tag=tag + "v")
            nc.vector.scalar_tensor_tensor(v_r[:], s1r[:], 1.0 / parts,
                                           msq[:], OP.mult, OP.subtract)
            sd_r = po.tile([128, T2 // 128], F32, tag=tag + "sd")
            nc.scalar.activation(sd_r[:], v_r[:], AF.Sqrt, bias=eps_sb[:parts if False else 128, 0:1])
            rs_r = po.tile([128, T2 // 128], F32, tag=tag + "rs")
            nc.vector.reciprocal(rs_r[:], sd_r[:])
            mu1 = po.tile([1, T2], F32, tag=tag + "mu1")
            rs1 = po.tile([1, T2], F32, tag=tag + "rs1")
            nc.sync.dma_start(mu1[:], m_r[:])
            nc.sync.dma_start(rs1[:], rs_r[:])
            mu_b = po.tile([parts, T2], F32, tag=tag + "mub")
            rs_b = po.tile([parts, T2], F32, tag=tag + "rsb")
            nc.gpsimd.partition_broadcast(mu_b[:], mu1[:])
            nc.gpsimd.partition_broadcast(rs_b[:], rs1[:])
            return mu_b, rs_b

        # out_norm (over Di) + gate
        mu_b, rs_b = pln(y_sb[:], Di, "a")
        t1 = po.tile([Di, T2], F32, tag="t1")
        nc.vector.tensor_tensor(t1[:], y_sb[:], mu_b[:], OP.subtract)
        t2 = po.tile([Di, T2], F32, tag="t2")
        nc.vector.tensor_tensor(t2[:], t1[:], rs_b[:], OP.mult)
        t3 = po.tile([Di, T2], F32, tag="t3")
        nc.vector.tensor_scalar(t3[:], t2[:], og_sb[:, 0:1], ob_sb[:, 0:1],
                                OP.mult, OP.add)
        yg = po.tile([Di, T2], F32, tag="yg")
        nc.vector.tensor_tensor(yg[:], t3[:], sz_sb[:], OP.mult)

        # out_proj + residual ;  "mm" psum tag shared/serialized
        x2 = po.tile([D, T2], F32, tag="x2")
        opps = ps.tile([128, T2], F32, tag="mm")
        for f in range(T2 // 512):
            fsl = slice(f * 512, (f + 1) * 512)
            nc.tensor.matmul(opps[0:D, fsl], op_sb[:], yg[:, fsl],
                             start=True, stop=True)
        nc.vector.tensor_tensor(x2[:], opps[0:D, :], x_sb[:], OP.add)

        # LN2 (over D) -> fc1 -> gelu -> fc2 -> + residual
        mu2, rs2 = pln(x2[:], D, "b")
        h1 = po.tile([D, T2], F32, tag="h1")
        nc.vector.tensor_tensor(h1[:], x2[:], mu2[:], OP.subtract)
        hn = po.tile([D, T2], F32, tag="hn")
        nc.vector.tensor_tensor(hn[:], h1[:], rs2[:], OP.mult)

        g1 = po.tile([128, 2, T2], F32, tag="g1")
        for mc in range(2):
            fp = ps.tile([128, T2], F32, tag="mm")
            for f in range(T2 // 512):
                fsl = slice(f * 512, (f + 1) * 512)
                nc.tensor.matmul(fp[:, fsl],
                                 fc1_sb[:, mc * 128:(mc + 1) * 128],
                                 hn[:, fsl], start=True, stop=True)
            nc.scalar.activation(g1[:, mc, :], fp[:],
                                 AF.Gelu_apprx_tanh, bias=b1_sb[:, mc:mc + 1])
        f2 = ps.tile([128, T2], F32, tag="mm")
        for f in range(T2 // 512):
            fsl = slice(f * 512, (f + 1) * 512)
            for mc in range(2):
                nc.tensor.matmul(f2[0:D, fsl], fc2_sb[:, mc, :],
                                 g1[:, mc, fsl],
                                 start=(mc == 0), stop=(mc == 1))
        o_sb = po.tile([D, T2], F32, tag="o")
        nc.vector.scalar_tensor_tensor(o_sb[:], f2[0:D, :], fc2b_sb[:, 0:1],
                                       x2[:], OP.add, OP.add)
        nc.sync.dma_start(out[:], o_sb[:])

    nc.compile()
    return nc


# ---------------------------------------------------------------- host side

_CACHE = {}


def _get_programs():
    if "nc1" not in _CACHE:
        _CACHE["nc1"] = build_launch1()
        _CACHE["nc2"] = build_launch2()
    return _CACHE["nc1"], _CACHE["nc2"]


def _prep_inmaps(inputs):
    f32 = lambda a: np.ascontiguousarray(np.asarray(a), dtype=np.float32)
    conv_w = f32(inputs["conv_w"])
    conv_b = f32(inputs["conv_b"])
    ln1_g, ln1_b = f32(inputs["ln1_g"]), f32(inputs["ln1_b"])
    in_proj_w = f32(inputs["in_proj_w"])
    dw_w_all = f32(inputs["conv_dw_w"])[:, 0]
    dw_b = f32(inputs["conv_dw_b"])
    x_proj_w = f32(inputs["x_proj_w"])
    dt_proj_w = f32(inputs["dt_proj_w"])
    dt_proj_b = f32(inputs["dt_proj_b"])
    A = np.exp(f32(inputs["A_logs"])).reshape(K, Di, N).astype(np.float32)
    Ds = f32(inputs["Ds"]).reshape(K, Di)

    Wp = (ln1_g[:, None] * in_proj_w).astype(np.float32)        # [64, 256]
    Wp_bf = Wp.astype(ml_dtypes.bfloat16)
    q = Wp.sum(0)
    bias_full = (ln1_b @ in_proj_w).astype(np.float32)          # [256]
    negq = np.ascontiguousarray(np.stack([-q[:Di], -q[Di:]], 1), np.float32)
    sel = np.zeros((128, 2), np.float32)
    sel[:D, 0] = 1.0
    sel[D:, 1] = 1.0
    ones1 = np.ones((1, 128), np.float32)

    Ps = _perms()
    x123 = [np.concatenate([f32(inputs["x1"])[b], f32(inputs["x2"])[b],
                            f32(inputs["x3"])[b]], 0).reshape(3 * C, L)
            for b in range(B)]

    bsel_np = np.zeros((R + 2 * N, N * 128), ml_dtypes.bfloat16)
    csel_np = np.zeros((R + 2 * N, N * 128), ml_dtypes.bfloat16)
    for n in range(N):
        bsel_np[R + n, n * 128:(n + 1) * 128] = 1.0
        csel_np[R + N + n, n * 128:(n + 1) * 128] = 1.0
    shared = {
        "bsel": bsel_np, "csel": csel_np,
        "convT": np.ascontiguousarray(conv_w.T),
        "conv_b": conv_b.reshape(D, 1).copy(),
        "sel": sel, "ones1": ones1, "Wp": Wp_bf, "negq": negq,
        "bias_z": bias_full[Di:].reshape(Di, 1).copy(),
    }
    in_maps = []
    for core in range(8):
        b, k = core // 4, core % 4
        dw_w = _permute_kernel(dw_w_all, k)
        wsum = dw_w.sum((1, 2))
        dwdiag = np.zeros((9, 128, 128), ml_dtypes.bfloat16)
        for t in range(9):
            np.fill_diagonal(dwdiag[t], dw_w[:, t // 3, t % 3])
        in_maps.append({
            **shared,
            "xin": np.ascontiguousarray(x123[b][:, Ps[k]]),
            "dwdiag": dwdiag,
            "bias_dw": (dw_b + bias_full[:Di] * wsum).reshape(Di, 1)
                        .astype(np.float32),
            "xprojT": np.ascontiguousarray(x_proj_w[k].T.astype(ml_dtypes.bfloat16)),
            "dtT": np.ascontiguousarray(dt_proj_w[k].T.astype(ml_dtypes.bfloat16)),
            "dtb": (-dt_proj_b[k]).reshape(Di, 1).copy(),
            "A_in": np.ascontiguousarray(A[k]),
            "Ds_in": Ds[k].reshape(Di, 1).copy(),
        })
    return in_maps, Ps


def _prep_inmaps2(inputs, y_merged, sz_full, x_full):
    f32 = lambda a: np.ascontiguousarray(np.asarray(a), dtype=np.float32)
    ln2_g, ln2_b = f32(inputs["ln2_g"]), f32(inputs["ln2_b"])
    fc1_w, fc1_b = f32(inputs["fc1_w"]), f32(inputs["fc1_b"])
    fc1p = (ln2_g[:, None] * fc1_w).astype(np.float32)
    bias1 = (ln2_b @ fc1_w + fc1_b).astype(np.float32)
    shared = {
        "ones128": np.ones((128, 1), np.float32),
        "onorm_g": f32(inputs["out_norm_g"]).reshape(Di, 1).copy(),
        "onorm_b": f32(inputs["out_norm_b"]).reshape(Di, 1).copy(),
        "oproj": f32(inputs["out_proj_w"]),
        "fc1p": fc1p,
        "bias1": np.ascontiguousarray(np.stack([bias1[:128], bias1[128:]], 1),
                                      np.float32),
        "fc2w": f32(inputs["fc2_w"]),
        "fc2b": f32(inputs["fc2_b"]).reshape(D, 1).copy(),
    }
    in_maps = []
    for core in range(8):
        b, sl = core // 4, slice((core % 4) * T2, (core % 4 + 1) * T2)
        in_maps.append({
            **shared,
            "y_in": np.ascontiguousarray(y_merged[b][:, sl]),
            "sz_in": np.ascontiguousarray(sz_full[b][:, sl]),
            "x_in": np.ascontiguousarray(x_full[b][:, sl]),
        })
    return in_maps


def kernel(**inputs):
    nc1, nc2 = _get_programs()
    in_maps, Ps = _prep_inmaps(inputs)
    res1 = run_bass_kernel_spmd(nc1, in_maps, list(range(8))).results

    y_merged = np.zeros((B, Di, L), np.float32)
    sz_full = [None] * B
    x_full = [None] * B
    for core in range(8):
        b, k = core // 4, core % 4
        y_merged[b][:, Ps[k]] += res1[core]["y_out"]
        if k == 0:
            sz_full[b] = res1[core]["sz_out"]
            x_full[b] = res1[core]["x_out"]

    in_maps2 = _prep_inmaps2(inputs, y_merged, sz_full, x_full)
    res2 = run_bass_kernel_spmd(nc2, in_maps2, list(range(8))).results

    out = np.zeros((B, D, L), np.float32)
    for core in range(8):
        b, sl = core // 4, slice((core % 4) * T2, (core % 4 + 1) * T2)
        out[b][:, sl] = res2[core]["out"]
    return out.reshape(B, D, H, W)


# revision 31
# speedup vs baseline: 1.1325x; 1.0560x over previous
"""Trainium2 Bass kernel for nn_Decoder_17489106830107 (VMamba VSSBlock decoder).

Sharding: one (batch, scan-direction) pair per core (B=2 x K=4 = 8 cores).
The host pre-permutes each core's inputs into that core's scan coordinate
order (transpose / 180-rotation of the image), so all 8 cores run ONE
identical SPMD program for launch 1 (conv1x1 -> LN -> in_proj -> depthwise
conv -> x_proj/dt -> 16 hardware linear-recurrence scans on the DVE).
The host then scatter-adds the 4 directional outputs per batch and an
8-way token-parallel launch 2 does the merge epilogue (out_norm, gating,
out_proj, MLP).
"""
import numpy as np
from contextlib import ExitStack

import concourse.bacc as bacc
import concourse.bass as bass
import concourse.mybir as mybir
import concourse.tile as tile
from concourse.bass_utils import run_bass_kernel_spmd
import ml_dtypes

F32 = mybir.dt.float32
F32R = mybir.dt.float32r
BF16 = mybir.dt.bfloat16
AF = mybir.ActivationFunctionType
OP = mybir.AluOpType

B, C, H, W = 2, 256, 64, 64
D = 64
Di = 128
N = 16
R = 4
K = 4
L = H * W          # 4096
LC = 1024          # scan chunk
NCH = L // LC
EPS = 1e-5
T2 = 1024          # launch-2 token slice per core


# ---------------------------------------------------------------- host prep

def _perms():
    ar = np.arange(L)
    p1 = (ar % 64) * 64 + ar // 64
    return [ar, p1, ar[::-1].copy(), p1[::-1].copy()]


def _permute_kernel(w, k):
    if k == 0:
        return w
    if k == 1:
        return w.transpose(0, 2, 1)
    if k == 2:
        return w[:, ::-1, ::-1]
    return w.transpose(0, 2, 1)[:, ::-1, ::-1]


# ---------------------------------------------------------------- launch 1

def _r(ap):
    return ap.bitcast(F32R)


def build_launch1():
    nc = bacc.Bacc("TRN2", target_bir_lowering=False, debug=False,
                   num_devices=8)

    def inp(name, shape):
        return nc.dram_tensor(name, shape, F32, kind="ExternalInput")

    xin = inp("xin", [3 * C, L])
    convT = inp("convT", [3 * C, D])
    conv_b = inp("conv_b", [D, 1])
    sel = inp("sel", [128, 2])
    ones1 = inp("ones1", [1, 128])
    Wp = nc.dram_tensor("Wp", [D, 2 * Di], BF16,
                        kind="ExternalInput")
    negq = inp("negq", [128, 2])
    bias_z = inp("bias_z", [128, 1])
    dwdiag = nc.dram_tensor("dwdiag", [9, 128, 128], BF16,
                            kind="ExternalInput")
    bias_dw = inp("bias_dw", [128, 1])
    xprojT = nc.dram_tensor("xprojT", [Di, R + 2 * N], BF16,
                            kind="ExternalInput")
    dtT = nc.dram_tensor("dtT", [R, Di], BF16,
                         kind="ExternalInput")
    dtb = inp("dtb", [Di, 1])
    A_in = inp("A_in", [Di, N])
    bsel = nc.dram_tensor("bsel", [R + 2 * N, N * 128], BF16,
                          kind="ExternalInput")
    csel = nc.dram_tensor("csel", [R + 2 * N, N * 128], BF16,
                          kind="ExternalInput")
    Ds_in = inp("Ds_in", [Di, 1])

    y_out = nc.dram_tensor("y_out", [Di, L], F32, kind="ExternalOutput")
    sz_out = nc.dram_tensor("sz_out", [Di, L], F32, kind="ExternalOutput")
    x_out = nc.dram_tensor("x_out", [D, L], F32, kind="ExternalOutput")

    with tile.TileContext(nc) as tc, ExitStack() as ctx:
        cpool = ctx.enter_context(tc.tile_pool(name="consts", bufs=1))
        main = ctx.enter_context(tc.tile_pool(name="main", bufs=1))

        # ---- const loads
        convT_sb = cpool.tile([128, 6, D], F32, tag="convT")
        nc.sync.dma_start(convT_sb[:], convT[:].rearrange("(c p) m -> p c m", p=128))
        conv_b_sb = cpool.tile([D, 1], F32, tag="convb")
        nc.sync.dma_start(conv_b_sb[:], conv_b[:])
        sel_sb = cpool.tile([128, 2], F32, tag="sel")
        nc.sync.dma_start(sel_sb[:], sel[:])
        ones1_sb = cpool.tile([1, 128], F32, tag="ones1")
        nc.sync.dma_start(ones1_sb[:], ones1[:])
        Wp_sb = cpool.tile([D, 2 * Di], BF16, tag="Wp")
        nc.sync.dma_start(Wp_sb[:], Wp[:])
        negq_sb = cpool.tile([128, 2], F32, tag="negq")
        nc.sync.dma_start(negq_sb[:], negq[:])
        bias_z_sb = cpool.tile([128, 1], F32, tag="biasz")
        nc.sync.dma_start(bias_z_sb[:], bias_z[:])
        dwdiag_sb = cpool.tile([128, 9, 128], BF16, tag="dwdiag")
        nc.sync.dma_start(dwdiag_sb[:], dwdiag[:].rearrange("t p f -> p t f"))
        bias_dw_sb = cpool.tile([128, 1], F32, tag="biasdw")
        nc.sync.dma_start(bias_dw_sb[:], bias_dw[:])
        xprojT_sb = cpool.tile([Di, R + 2 * N], BF16, tag="xprojT")
        nc.sync.dma_start(xprojT_sb[:], xprojT[:])
        dtT_sb = cpool.tile([R, Di], BF16, tag="dtT")
        nc.sync.dma_start(dtT_sb[:], dtT[:])
        dtb_sb = cpool.tile([Di, 1], F32, tag="dtb")
        nc.sync.dma_start(dtb_sb[:], dtb[:])
        A_sb = cpool.tile([Di, N], F32, tag="A")
        nc.sync.dma_start(A_sb[:], A_in[:])
        Ds_sb = cpool.tile([Di, 1], F32, tag="Ds")
        nc.sync.dma_start(Ds_sb[:], Ds_in[:])
        eps_sb = cpool.tile([128, 1], F32, tag="eps")
        nc.vector.memset(eps_sb[:], EPS)
        bsel_sb = cpool.tile([R + 2 * N, N * 128], BF16, tag="bsel")
        nc.sync.dma_start(bsel_sb[:], bsel[:])
        csel_sb = cpool.tile([R + 2 * N, N * 128], BF16, tag="csel")
        nc.sync.dma_start(csel_sb[:], csel[:])

        # ---- persistent activations
        xc_sb = main.tile([Di, L], BF16, tag="xc")
        xdbl_bf = main.tile([R + 2 * N, L], BF16, tag="xdblbf")
        delta_sb = main.tile([Di, L], F32, tag="delta")
        du_sb = main.tile([Di, L], F32, tag="du")
        carry_sb = main.tile([Di, N], F32, tag="carry")

        with tc.tile_pool(name="imgp", bufs=1) as imgp:
            img = imgp.tile([Di, 66 * 66], BF16, tag="img")
            nc.gpsimd.memset(img[:], 0.0)
            img3 = img[:].rearrange("p (h w) -> p h w", h=66)

            with tc.tile_pool(name="p1", bufs=1) as p1, \
                 tc.tile_pool(name="p1x", bufs=2) as p1x:
                lnin = p1.tile([128, L], F32, tag="lnin")

                # conv1x1: psum[f] accumulates over 6 channel chunks
                with tc.tile_pool(name="ps_conv", bufs=1,
                                  space="PSUM") as ps_conv:
                    cps = [ps_conv.tile([D, 512], F32, tag=f"cps{f}",
                                        name=f"cps{f}")
                           for f in range(8)]
                    for c in range(6):
                        xin_c = p1x.tile([128, L], F32, tag="xin")
                        nc.sync.dma_start(xin_c[:],
                                          xin[:][c * 128:(c + 1) * 128, :])
                        for f in range(8):
                            nc.tensor.matmul(cps[f][:], convT_sb[:, c, :],
                                             xin_c[:, f * 512:(f + 1) * 512],
                                             start=(c == 0), stop=(c == 5))
                    for f in range(8):
                        nc.scalar.activation(lnin[0:D, f * 512:(f + 1) * 512],
                                             cps[f][:], AF.Identity,
                                             bias=conv_b_sb[:, 0:1])
                nc.sync.dma_start(x_out[:], lnin[0:D, :])
                lnin_bf = p1.tile([D, L], BF16, tag="lninbf")
                nc.scalar.copy(lnin_bf[:], lnin[0:D, :])

                # ---- LN1 stats
                nc.scalar.square(lnin[D:2 * D, :], lnin[0:D, :])
                mu_b = p1.tile([128, L], F32, tag="mu_b")
                rs_b = p1.tile([128, L], F32, tag="rs_b")
                st0_sb = mu_b
                st1_sb = rs_b
                with tc.tile_pool(name="ps_st", bufs=1, space="PSUM") as ps_st:
                    for hh in range(2):
                        hsl = slice(hh * 2048, (hh + 1) * 2048)
                        st0 = ps_st.tile([1, L // 2], F32, tag="st0",
                                         name="st0")
                        st1 = ps_st.tile([1, L // 2], F32, tag="st1",
                                         name="st1")
                        for f in range(4):
                            fsl = slice(hh * 2048 + f * 512,
                                        hh * 2048 + (f + 1) * 512)
                            psl = slice(f * 512, (f + 1) * 512)
                            nc.tensor.matmul(st0[:, psl], sel_sb[:, 0:1],
                                             lnin[:, fsl], start=True,
                                             stop=True)
                            nc.tensor.matmul(st1[:, psl], sel_sb[:, 1:2],
                                             lnin[:, fsl], start=True,
                                             stop=True)
                        nc.scalar.copy(st0_sb[0:1, hsl], st0[:])
                        nc.scalar.copy(st1_sb[0:1, hsl], st1[:])

                s0r = p1.tile([128, 32], F32, tag="s0r")
                s1r = p1.tile([128, 32], F32, tag="s1r")
                nc.sync.dma_start(s0r[:], st0_sb[0:1, :])
                nc.sync.dma_start(s1r[:], st1_sb[0:1, :])
                m_r = p1.tile([128, 32], F32, tag="m_r")
                nc.scalar.mul(m_r[:], s0r[:], 1.0 / D)
                msq = p1.tile([128, 32], F32, tag="msq")
                nc.scalar.square(msq[:], m_r[:])
                v_r = p1.tile([128, 32], F32, tag="v_r")
                nc.vector.scalar_tensor_tensor(v_r[:], s1r[:], 1.0 / D, msq[:],
                                               OP.mult, OP.subtract)
                sd_r = p1.tile([128, 32], F32, tag="sd_r")
                nc.scalar.activation(sd_r[:], v_r[:], AF.Sqrt, bias=eps_sb[:, 0:1])
                rs_r = p1.tile([128, 32], F32, tag="rs_r")
                nc.vector.reciprocal(rs_r[:], sd_r[:])
                nc.sync.dma_start(mu_b[0:1, :], m_r[:])
                nc.sync.dma_start(rs_b[0:1, :], rs_r[:])
                nc.gpsimd.partition_broadcast(mu_b[:], mu_b[0:1, :])
                nc.gpsimd.partition_broadcast(rs_b[:], rs_b[0:1, :])

                # ---- in_proj + LN fixup
                with tc.tile_pool(name="ps_ip", bufs=4, space="PSUM") as ps_ip, \
                     tc.tile_pool(name="fix", bufs=3) as fix:
                    for mc in range(2):
                        for f in range(8):
                            fsl = slice(f * 512, (f + 1) * 512)
                            pp = ps_ip.tile([128, 512], F32, tag="ipps")
                            nc.tensor.matmul(pp[:],
                                             Wp_sb[:, mc * 128:(mc + 1) * 128],
                                             lnin_bf[:, fsl],
                                             start=True, stop=True)
                            t1 = fix.tile([128, 512], F32, tag="t1")
                            nc.vector.scalar_tensor_tensor(
                                t1[:], mu_b[:, fsl], negq_sb[:, mc:mc + 1],
                                pp[:], OP.mult, OP.add)
                            if mc == 0:
                                r0 = f * 8
                                nc.vector.tensor_tensor(
                                    img3[:, 1 + r0:1 + r0 + 8, 1:65],
                                    t1[:].rearrange("p (r w) -> p r w", r=8),
                                    rs_b[:, fsl].rearrange("p (r w) -> p r w",
                                                           r=8),
                                    OP.mult)
                            else:
                                t2 = fix.tile([128, 512], F32, tag="t2")
                                nc.vector.tensor_tensor(t2[:], t1[:],
                                                        rs_b[:, fsl], OP.mult)
                                szt = fix.tile([128, 512], F32, tag="szt")
                                nc.scalar.activation(szt[:], t2[:],
                                                     AF.Silu,
                                                     bias=bias_z_sb[:, 0:1])
                                nc.sync.dma_start(sz_out[:][:, fsl], szt[:])

            # ---- interleaved per-chunk pipeline: dwconv -> x_proj ->
            # delta -> scans, so chunk c's scans overlap chunk c+1's prep
            dts_sb = main.tile([R, L], BF16, tag="dts")
            with tc.tile_pool(name="ps_dw", bufs=1, space="PSUM") as ps_dw, \
                 tc.tile_pool(name="ps_xp", bufs=1, space="PSUM") as ps_xp, \
                 tc.tile_pool(name="ps_dt", bufs=1, space="PSUM") as ps_dt, \
                 tc.tile_pool(name="ps_bb", bufs=1, space="PSUM") as ps_bb, \
                 tc.tile_pool(name="nl", bufs=3) as nl, \
                 tc.tile_pool(name="yp", bufs=2) as yp:
            
                for c in range(NCH):
                    csl = slice(c * LC, (c + 1) * LC)
                    for f in range(2 * c, 2 * c + 2):
                        fsl = slice(f * 512, (f + 1) * 512)
                        r0 = f * 8
                        dps = ps_dw.tile([128, 512], F32, tag="dwps")
                        for t in range(9):
                            di_, dj = t // 3, t % 3
                            nc.tensor.matmul(
                                dps[:], dwdiag_sb[:, t, :],
                                img3[:, r0 + di_:r0 + di_ + 8, dj:dj + 64],
                                start=(t == 0), stop=(t == 8))
                        nc.scalar.activation(xc_sb[:, fsl], dps[:], AF.Silu,
                                             bias=bias_dw_sb[:, 0:1])
                        xps = ps_xp.tile([R + 2 * N, 512], F32, tag="xpps")
                        nc.tensor.matmul(xps[:], xprojT_sb[:], xc_sb[:, fsl],
                                         start=True, stop=True)
                        nc.scalar.copy(xdbl_bf[:, fsl], xps[:])
                        nc.scalar.copy(dts_sb[:, fsl], xps[0:R, :])
                        dtps = ps_dt.tile([Di, 512], F32, tag="dtps")
                        nc.tensor.matmul(dtps[:], dtT_sb[:], dts_sb[:, fsl],
                                         start=True, stop=True)
                        nc.scalar.activation(delta_sb[:, fsl], dtps[:],
                                             AF.Sigmoid,
                                             bias=dtb_sb[:, 0:1], scale=-1.0)
                    nc.scalar.activation(delta_sb[:, csl], delta_sb[:, csl],
                                         AF.Ln)
                    nc.vector.scalar_tensor_tensor(du_sb[:, csl],
                                                   delta_sb[:, csl],
                                                   -1.0, xc_sb[:, csl],
                                                   OP.mult, OP.mult)
                    y_acc = yp.tile([Di, LC], F32, tag="yacc")
                    for n in range(N):
                        bb = ps_bb.tile([128, LC], F32, tag="bb")
                        for j in range(LC // 512):
                            nc.tensor.matmul(
                                bb[:, j * 512:(j + 1) * 512],
                                bsel_sb[:, n * 128:(n + 1) * 128],
                                xdbl_bf[:, c * LC + j * 512:
                                        c * LC + (j + 1) * 512],
                                start=True, stop=True)
                        cb = ps_bb.tile([128, LC], F32, tag="cb")
                        for j in range(LC // 512):
                            nc.tensor.matmul(
                                cb[:, j * 512:(j + 1) * 512],
                                csel_sb[:, n * 128:(n + 1) * 128],
                                xdbl_bf[:, c * LC + j * 512:
                                        c * LC + (j + 1) * 512],
                                start=True, stop=True)
                        da = nl.tile([Di, LC], F32, tag="da")
                        nc.scalar.activation(da[:], delta_sb[:, csl], AF.Exp,
                                             scale=A_sb[:, n:n + 1])
                        dbu = nl.tile([Di, LC], F32, tag="dbu")
                        nc.vector.tensor_tensor(dbu[:], du_sb[:, csl], bb[:],
                                                OP.mult)
                        h = nl.tile([Di, LC], F32, tag="h")
                        nc.vector.tensor_tensor_scan(
                            h[:], da[:], dbu[:],
                            0.0 if c == 0 else carry_sb[:, n:n + 1],
                            OP.mult, OP.add)
                        if c < NCH - 1:
                            nc.vector.tensor_copy(carry_sb[:, n:n + 1],
                                                  h[:, LC - 1:LC])
                        if n == 0:
                            nc.vector.tensor_tensor(y_acc[:], h[:], cb[:],
                                                    OP.mult)
                        else:
                            tmp = nl.tile([Di, LC], F32, tag="tmp")
                            nc.vector.tensor_tensor(tmp[:], h[:], cb[:],
                                                    OP.mult)
                            nc.gpsimd.tensor_tensor(y_acc[:], y_acc[:],
                                                    tmp[:], OP.add)
                    y_f = yp.tile([Di, LC], F32, tag="yout")
                    nc.vector.scalar_tensor_tensor(y_f[:], xc_sb[:, csl],
                                                   Ds_sb[:, 0:1], y_acc[:],
                                                   OP.mult, OP.add)
                    nc.sync.dma_start(y_out[:][:, csl], y_f[:])

    nc.compile()
    return nc


def _finish(nc):
    return nc


# ---------------------------------------------------------------- launch 2

def build_launch2():
    nc = bacc.Bacc("TRN2", target_bir_lowering=False, debug=False,
                   num_devices=8)

    def inp(name, shape):
        return nc.dram_tensor(name, shape, F32, kind="ExternalInput")

    y_in = nc.dram_tensor("y_in", [Di, T2], BF16, kind="ExternalInput")
    sz_in = nc.dram_tensor("sz_in", [Di, T2], BF16, kind="ExternalInput")
    x_in = inp("x_in", [D, T2])
    ones128 = inp("ones128", [128, 1])
    onorm_g = inp("onorm_g", [Di, 1])
    onorm_b = inp("onorm_b", [Di, 1])
    oproj = inp("oproj", [Di, D])
    fc1p = inp("fc1p", [D, 2 * Di])
    bias1 = inp("bias1", [128, 2])
    fc2w = inp("fc2w", [2 * Di, D])
    fc2b = inp("fc2b", [D, 1])
    out = nc.dram_tensor("out", [D, T2], F32, kind="ExternalOutput")

    with tile.TileContext(nc) as tc, ExitStack() as ctx:
        po = ctx.enter_context(tc.tile_pool(name="main", bufs=1))
        ps = ctx.enter_context(tc.tile_pool(name="psum", bufs=1, space="PSUM"))

        y_sb = po.tile([Di, T2], BF16, tag="y")
        nc.sync.dma_start(y_sb[:], y_in[:])
        sz_sb = po.tile([Di, T2], BF16, tag="sz")
        nc.sync.dma_start(sz_sb[:], sz_in[:])
        x_sb = po.tile([D, T2], F32, tag="x")
        nc.sync.dma_start(x_sb[:], x_in[:])
        ones_sb = po.tile([128, 1], F32, tag="ones")
        nc.sync.dma_start(ones_sb[:], ones128[:])
        ones_bf = po.tile([128, 1], BF16, tag="onesbf")
        nc.vector.memset(ones_bf[:], 1.0)
        og_sb = po.tile([Di, 1], F32, tag="og")
        nc.sync.dma_start(og_sb[:], onorm_g[:])
        ob_sb = po.tile([Di, 1], F32, tag="ob")
        nc.sync.dma_start(ob_sb[:], onorm_b[:])
        op_sb = po.tile([Di, D], F32, tag="oproj")
        nc.sync.dma_start(op_sb[:], oproj[:])
        fc1_sb = po.tile([D, 2 * Di], F32, tag="fc1")
        nc.sync.dma_start(fc1_sb[:], fc1p[:])
        b1_sb = po.tile([128, 2], F32, tag="b1")
        nc.sync.dma_start(b1_sb[:], bias1[:])
        fc2_sb = po.tile([128, 2, D], F32, tag="fc2")
        nc.sync.dma_start(fc2_sb[:], fc2w[:].rearrange("(c p) m -> p c m", p=128))
        fc2b_sb = po.tile([D, 1], F32, tag="fc2b")
        nc.sync.dma_start(fc2b_sb[:], fc2b[:])
        eps_sb = po.tile([128, 1], F32, tag="eps")
        nc.vector.memset(eps_sb[:], EPS)

        def pln(src, parts, tag, dt=F32, ones_t=None):
            """LayerNorm stats over the partition dim of src [parts, T2];
            returns broadcast (mu_b, rs_b) [parts, T2] tiles."""
            ones_t = ones_sb if ones_t is None else ones_t
            sq = po.tile([parts, T2], dt, tag=tag + "sq")
            nc.scalar.square(sq[:], src)
            st0_sb = po.tile([1, T2], F32, tag=tag + "st0sb")
            st1_sb = po.tile([1, T2], F32, tag=tag + "st1sb")
            with tc.tile_pool(name=tag + "ps_st", bufs=1,
                              space="PSUM") as ps_st:
                st0 = ps_st.tile([1, T2], F32, tag="st0")
                st1 = ps_st.tile([1, T2], F32, tag="st1")
                for f in range(T2 // 512):
                    fsl = slice(f * 512, (f + 1) * 512)
                    nc.tensor.matmul(st0[:, fsl], ones_t[0:parts, :],
                                     src[:, fsl], start=True, stop=True)
                    nc.tensor.matmul(st1[:, fsl], ones_t[0:parts, :],
                                     sq[:, fsl], start=True, stop=True)
                nc.scalar.copy(st0_sb[:], st0[:])
                nc.scalar.copy(st1_sb[:], st1[:])
            s0r = po.tile([128, T2 // 128], F32, tag=tag + "s0r")
            s1r = po.tile([128, T2 // 128], F32, tag=tag + "s1r")
            nc.sync.dma_start(s0r[:], st0_sb[:])
            nc.sync.dma_start(s1r[:], st1_sb[:])
            m_r = po.tile([128, T2 // 128], F32, tag=tag + "m")
            nc.scalar.mul(m_r[:], s0r[:], 1.0 / parts)
            msq = po.tile([128, T2 // 128], F32, tag=tag + "msq")
            nc.scalar.square(msq[:], m_r[:])
            v_r = po.tile([128, T2 // 128], F32, tag=tag + "v")
            nc.vector.scalar_tensor_tensor(v_r[:], s1r[:], 1.0 / parts,
                                           msq[:], OP.mult, OP.subtract)
            sd_r = po.tile([128, T2 // 128], F32, tag=tag + "sd")
            nc.scalar.activation(sd_r[:], v_r[:], AF.Sqrt, bias=eps_sb[:parts if False else 128, 0:1])
            rs_r = po.tile([128, T2 // 128], F32, tag=tag + "rs")
            nc.vector.reciprocal(rs_r[:], sd_r[:])
            mu1 = po.tile([1, T2], F32, tag=tag + "mu1")
            rs1 = po.tile([1, T2], F32, tag=tag + "rs1")
            nc.sync.dma_start(mu1[:], m_r[:])
            nc.sync.dma_start(rs1[:], rs_r[:])
            mu_b = po.tile([parts, T2], F32, tag=tag + "mub")
            rs_b = po.tile([parts, T2], F32, tag=tag + "rsb")
            nc.gpsimd.partition_broadcast(mu_b[:], mu1[:])
            nc.gpsimd.partition_broadcast(rs_b[:], rs1[:])
            return mu_b, rs_b

        # out_norm (over Di) + gate
        mu_b, rs_b = pln(y_sb[:], Di, "a", dt=BF16, ones_t=ones_bf)
        t1 = po.tile([Di, T2], F32, tag="t1")
        nc.vector.tensor_tensor(t1[:], y_sb[:], mu_b[:], OP.subtract)
        t2 = po.tile([Di, T2], F32, tag="t2")
        nc.vector.tensor_tensor(t2[:], t1[:], rs_b[:], OP.mult)
        t3 = po.tile([Di, T2], F32, tag="t3")
        nc.vector.tensor_scalar(t3[:], t2[:], og_sb[:, 0:1], ob_sb[:, 0:1],
                                OP.mult, OP.add)
        yg = po.tile([Di, T2], F32, tag="yg")
        nc.vector.tensor_tensor(yg[:], t3[:], sz_sb[:], OP.mult)

        # out_proj + residual ;  "mm" psum tag shared/serialized
        x2 = po.tile([D, T2], F32, tag="x2")
        opps = ps.tile([128, T2], F32, tag="mm")
        for f in range(T2 // 512):
            fsl = slice(f * 512, (f + 1) * 512)
            nc.tensor.matmul(opps[0:D, fsl], op_sb[:], yg[:, fsl],
                             start=True, stop=True)
        nc.vector.tensor_tensor(x2[:], opps[0:D, :], x_sb[:], OP.add)

        # LN2 (over D) -> fc1 -> gelu -> fc2 -> + residual
        mu2, rs2 = pln(x2[:], D, "b")
        h1 = po.tile([D, T2], F32, tag="h1")
        nc.vector.tensor_tensor(h1[:], x2[:], mu2[:], OP.subtract)
        hn = po.tile([D, T2], F32, tag="hn")
        nc.vector.tensor_tensor(hn[:], h1[:], rs2[:], OP.mult)

        g1 = po.tile([128, 2, T2], F32, tag="g1")
        for mc in range(2):
            fp = ps.tile([128, T2], F32, tag="mm")
            for f in range(T2 // 512):
                fsl = slice(f * 512, (f + 1) * 512)
                nc.tensor.matmul(fp[:, fsl],
                                 fc1_sb[:, mc * 128:(mc + 1) * 128],
                                 hn[:, fsl], start=True, stop=True)
            nc.scalar.activation(g1[:, mc, :], fp[:],
                                 AF.Gelu_apprx_tanh, bias=b1_sb[:, mc:mc + 1])
        f2 = ps.tile([128, T2], F32, tag="mm")
        for f in range(T2 // 512):
            fsl = slice(f * 512, (f + 1) * 512)
            for mc in range(2):
                nc.tensor.matmul(f2[0:D, fsl], fc2_sb[:, mc, :],
                                 g1[:, mc, fsl],
                                 start=(mc == 0), stop=(mc == 1))
        o_sb = po.tile([D, T2], F32, tag="o")
        nc.vector.scalar_tensor_tensor(o_sb[:], f2[0:D, :], fc2b_sb[:, 0:1],
                                       x2[:], OP.add, OP.add)
        nc.sync.dma_start(out[:], o_sb[:])

    nc.compile()
    return nc


# ---------------------------------------------------------------- host side

_CACHE = {}


def _get_programs():
    if "nc1" not in _CACHE:
        _CACHE["nc1"] = build_launch1()
        _CACHE["nc2"] = build_launch2()
    return _CACHE["nc1"], _CACHE["nc2"]


def _prep_inmaps(inputs):
    f32 = lambda a: np.ascontiguousarray(np.asarray(a), dtype=np.float32)
    conv_w = f32(inputs["conv_w"])
    conv_b = f32(inputs["conv_b"])
    ln1_g, ln1_b = f32(inputs["ln1_g"]), f32(inputs["ln1_b"])
    in_proj_w = f32(inputs["in_proj_w"])
    dw_w_all = f32(inputs["conv_dw_w"])[:, 0]
    dw_b = f32(inputs["conv_dw_b"])
    x_proj_w = f32(inputs["x_proj_w"])
    dt_proj_w = f32(inputs["dt_proj_w"])
    dt_proj_b = f32(inputs["dt_proj_b"])
    A = np.exp(f32(inputs["A_logs"])).reshape(K, Di, N).astype(np.float32)
    Ds = f32(inputs["Ds"]).reshape(K, Di)

    Wp = (ln1_g[:, None] * in_proj_w).astype(np.float32)        # [64, 256]
    Wp_bf = Wp.astype(ml_dtypes.bfloat16)
    q = Wp.sum(0)
    bias_full = (ln1_b @ in_proj_w).astype(np.float32)          # [256]
    negq = np.ascontiguousarray(np.stack([-q[:Di], -q[Di:]], 1), np.float32)
    sel = np.zeros((128, 2), np.float32)
    sel[:D, 0] = 1.0
    sel[D:, 1] = 1.0
    ones1 = np.ones((1, 128), np.float32)

    Ps = _perms()
    x123 = [np.concatenate([f32(inputs["x1"])[b], f32(inputs["x2"])[b],
                            f32(inputs["x3"])[b]], 0).reshape(3 * C, L)
            for b in range(B)]

    bsel_np = np.zeros((R + 2 * N, N * 128), ml_dtypes.bfloat16)
    csel_np = np.zeros((R + 2 * N, N * 128), ml_dtypes.bfloat16)
    for n in range(N):
        bsel_np[R + n, n * 128:(n + 1) * 128] = 1.0
        csel_np[R + N + n, n * 128:(n + 1) * 128] = 1.0
    shared = {
        "bsel": bsel_np, "csel": csel_np,
        "convT": np.ascontiguousarray(conv_w.T),
        "conv_b": conv_b.reshape(D, 1).copy(),
        "sel": sel, "ones1": ones1, "Wp": Wp_bf, "negq": negq,
        "bias_z": bias_full[Di:].reshape(Di, 1).copy(),
    }
    in_maps = []
    for core in range(8):
        b, k = core // 4, core % 4
        dw_w = _permute_kernel(dw_w_all, k)
        wsum = dw_w.sum((1, 2))
        dwdiag = np.zeros((9, 128, 128), ml_dtypes.bfloat16)
        for t in range(9):
            np.fill_diagonal(dwdiag[t], dw_w[:, t // 3, t % 3])
        in_maps.append({
            **shared,
            "xin": np.ascontiguousarray(x123[b][:, Ps[k]]),
            "dwdiag": dwdiag,
            "bias_dw": (dw_b + bias_full[:Di] * wsum).reshape(Di, 1)
                        .astype(np.float32),
            "xprojT": np.ascontiguousarray(x_proj_w[k].T.astype(ml_dtypes.bfloat16)),
            "dtT": np.ascontiguousarray(dt_proj_w[k].T.astype(ml_dtypes.bfloat16)),
            "dtb": (-dt_proj_b[k]).reshape(Di, 1).copy(),
            "A_in": np.ascontiguousarray(A[k]),
            "Ds_in": Ds[k].reshape(Di, 1).copy(),
        })
    return in_maps, Ps


def _prep_inmaps2(inputs, y_merged, sz_full, x_full):
    f32 = lambda a: np.ascontiguousarray(np.asarray(a), dtype=np.float32)
    ln2_g, ln2_b = f32(inputs["ln2_g"]), f32(inputs["ln2_b"])
    fc1_w, fc1_b = f32(inputs["fc1_w"]), f32(inputs["fc1_b"])
    fc1p = (ln2_g[:, None] * fc1_w).astype(np.float32)
    bias1 = (ln2_b @ fc1_w + fc1_b).astype(np.float32)
    shared = {
        "ones128": np.ones((128, 1), np.float32),
        "onorm_g": f32(inputs["out_norm_g"]).reshape(Di, 1).copy(),
        "onorm_b": f32(inputs["out_norm_b"]).reshape(Di, 1).copy(),
        "oproj": f32(inputs["out_proj_w"]),
        "fc1p": fc1p,
        "bias1": np.ascontiguousarray(np.stack([bias1[:128], bias1[128:]], 1),
                                      np.float32),
        "fc2w": f32(inputs["fc2_w"]),
        "fc2b": f32(inputs["fc2_b"]).reshape(D, 1).copy(),
    }
    in_maps = []
    for core in range(8):
        b, sl = core // 4, slice((core % 4) * T2, (core % 4 + 1) * T2)
        in_maps.append({
            **shared,
            "y_in": np.ascontiguousarray(
                y_merged[b][:, sl].astype(ml_dtypes.bfloat16)),
            "sz_in": np.ascontiguousarray(
                sz_full[b][:, sl].astype(ml_dtypes.bfloat16)),
            "x_in": np.ascontiguousarray(x_full[b][:, sl]),
        })
    return in_maps


def kernel(**inputs):
    nc1, nc2 = _get_programs()
    in_maps, Ps = _prep_inmaps(inputs)
    res1 = run_bass_kernel_spmd(nc1, in_maps, list(range(8))).results

    y_merged = np.zeros((B, Di, L), np.float32)
    sz_full = [None] * B
    x_full = [None] * B
    for core in range(8):
        b, k = core // 4, core % 4
        y_merged[b][:, Ps[k]] += res1[core]["y_out"]
        if k == 0:
            sz_full[b] = res1[core]["sz_out"]
            x_full[b] = res1[core]["x_out"]

    in_maps2 = _prep_inmaps2(inputs, y_merged, sz_full, x_full)
    res2 = run_bass_kernel_spmd(nc2, in_maps2, list(range(8))).results

    out = np.zeros((B, D, L), np.float32)
    for core in range(8):
        b, sl = core // 4, slice((core % 4) * T2, (core % 4 + 1) * T2)
        out[b][:, sl] = res2[core]["out"]
    return out.reshape(B, D, H, W)


# revision 34
# speedup vs baseline: 1.1562x; 1.0209x over previous
"""Trainium2 Bass kernel for nn_Decoder_17489106830107 (VMamba VSSBlock decoder).

Sharding: one (batch, scan-direction) pair per core (B=2 x K=4 = 8 cores).
The host pre-permutes each core's inputs into that core's scan coordinate
order (transpose / 180-rotation of the image), so all 8 cores run ONE
identical SPMD program for launch 1 (conv1x1 -> LN -> in_proj -> depthwise
conv -> x_proj/dt -> 16 hardware linear-recurrence scans on the DVE).
The host then scatter-adds the 4 directional outputs per batch and an
8-way token-parallel launch 2 does the merge epilogue (out_norm, gating,
out_proj, MLP).
"""
import numpy as np
from contextlib import ExitStack

import concourse.bacc as bacc
import concourse.bass as bass
import concourse.mybir as mybir
import concourse.tile as tile
from concourse.bass_utils import run_bass_kernel_spmd
import ml_dtypes

F32 = mybir.dt.float32
F32R = mybir.dt.float32r
BF16 = mybir.dt.bfloat16
AF = mybir.ActivationFunctionType
OP = mybir.AluOpType

B, C, H, W = 2, 256, 64, 64
D = 64
Di = 128
N = 16
R = 4
K = 4
L = H * W          # 4096
LC = 1024          # scan chunk
NCH = L // LC
EPS = 1e-5
T2 = 1024          # launch-2 token slice per core


# ---------------------------------------------------------------- host prep

def _perms():
    ar = np.arange(L)
    p1 = (ar % 64) * 64 + ar // 64
    return [ar, p1, ar[::-1].copy(), p1[::-1].copy()]


def _permute_kernel(w, k):
    if k == 0:
        return w
    if k == 1:
        return w.transpose(0, 2, 1)
    if k == 2:
        return w[:, ::-1, ::-1]
    return w.transpose(0, 2, 1)[:, ::-1, ::-1]


# ---------------------------------------------------------------- launch 1

def _r(ap):
    return ap.bitcast(F32R)


def build_launch1():
    nc = bacc.Bacc("TRN2", target_bir_lowering=False, debug=False,
                   num_devices=8)

    def inp(name, shape):
        return nc.dram_tensor(name, shape, F32, kind="ExternalInput")

    xin = inp("xin", [3 * C, L])
    convT = inp("convT", [3 * C, D])
    conv_b = inp("conv_b", [D, 1])
    sel = inp("sel", [128, 2])
    ones1 = inp("ones1", [1, 128])
    Wp = nc.dram_tensor("Wp", [D, 2 * Di], BF16,
                        kind="ExternalInput")
    negq = inp("negq", [128, 2])
    bias_z = inp("bias_z", [128, 1])
    dwdiag = nc.dram_tensor("dwdiag", [9, 128, 128], BF16,
                            kind="ExternalInput")
    bias_dw = inp("bias_dw", [128, 1])
    xprojT = nc.dram_tensor("xprojT", [Di, R + 2 * N], BF16,
                            kind="ExternalInput")
    dtT = nc.dram_tensor("dtT", [R, Di], BF16,
                         kind="ExternalInput")
    dtb = inp("dtb", [Di, 1])
    A_in = inp("A_in", [Di, N])
    bsel = nc.dram_tensor("bsel", [R + 2 * N, N * 128], BF16,
                          kind="ExternalInput")
    csel = nc.dram_tensor("csel", [R + 2 * N, N * 128], BF16,
                          kind="ExternalInput")
    Ds_in = inp("Ds_in", [Di, 1])

    y_out = nc.dram_tensor("y_out", [Di, L], F32, kind="ExternalOutput")
    sz_out = nc.dram_tensor("sz_out", [Di, L], F32, kind="ExternalOutput")
    x_out = nc.dram_tensor("x_out", [D, L], F32, kind="ExternalOutput")

    with tile.TileContext(nc) as tc, ExitStack() as ctx:
        cpool = ctx.enter_context(tc.tile_pool(name="consts", bufs=1))
        main = ctx.enter_context(tc.tile_pool(name="main", bufs=1))

        # ---- const loads
        convT_sb = cpool.tile([128, 6, D], F32, tag="convT")
        nc.sync.dma_start(convT_sb[:], convT[:].rearrange("(c p) m -> p c m", p=128))
        conv_b_sb = cpool.tile([D, 1], F32, tag="convb")
        nc.sync.dma_start(conv_b_sb[:], conv_b[:])
        sel_sb = cpool.tile([128, 2], F32, tag="sel")
        nc.sync.dma_start(sel_sb[:], sel[:])
        ones1_sb = cpool.tile([1, 128], F32, tag="ones1")
        nc.sync.dma_start(ones1_sb[:], ones1[:])
        Wp_sb = cpool.tile([D, 2 * Di], BF16, tag="Wp")
        nc.sync.dma_start(Wp_sb[:], Wp[:])
        negq_sb = cpool.tile([128, 2], F32, tag="negq")
        nc.sync.dma_start(negq_sb[:], negq[:])
        bias_z_sb = cpool.tile([128, 1], F32, tag="biasz")
        nc.sync.dma_start(bias_z_sb[:], bias_z[:])
        dwdiag_sb = cpool.tile([128, 9, 128], BF16, tag="dwdiag")
        nc.sync.dma_start(dwdiag_sb[:], dwdiag[:].rearrange("t p f -> p t f"))
        bias_dw_sb = cpool.tile([128, 1], F32, tag="biasdw")
        nc.sync.dma_start(bias_dw_sb[:], bias_dw[:])
        xprojT_sb = cpool.tile([Di, R + 2 * N], BF16, tag="xprojT")
        nc.sync.dma_start(xprojT_sb[:], xprojT[:])
        dtT_sb = cpool.tile([R, Di], BF16, tag="dtT")
        nc.sync.dma_start(dtT_sb[:], dtT[:])
        dtb_sb = cpool.tile([Di, 1], F32, tag="dtb")
        nc.sync.dma_start(dtb_sb[:], dtb[:])
        A_sb = cpool.tile([Di, N], F32, tag="A")
        nc.sync.dma_start(A_sb[:], A_in[:])
        Ds_sb = cpool.tile([Di, 1], F32, tag="Ds")
        nc.sync.dma_start(Ds_sb[:], Ds_in[:])
        eps_sb = cpool.tile([128, 1], F32, tag="eps")
        nc.vector.memset(eps_sb[:], EPS)
        bsel_sb = cpool.tile([R + 2 * N, N * 128], BF16, tag="bsel")
        nc.sync.dma_start(bsel_sb[:], bsel[:])
        csel_sb = cpool.tile([R + 2 * N, N * 128], BF16, tag="csel")
        nc.sync.dma_start(csel_sb[:], csel[:])

        # ---- persistent activations
        xc_sb = main.tile([Di, L], BF16, tag="xc")
        xdbl_bf = main.tile([R + 2 * N, L], BF16, tag="xdblbf")
        delta_sb = main.tile([Di, L], F32, tag="delta")
        du_sb = main.tile([Di, L], F32, tag="du")
        carry_sb = main.tile([Di, N], F32, tag="carry")

        with tc.tile_pool(name="imgp", bufs=1) as imgp:
            img = imgp.tile([Di, 66 * 66], BF16, tag="img")
            nc.gpsimd.memset(img[:], 0.0)
            img3 = img[:].rearrange("p (h w) -> p h w", h=66)

            with tc.tile_pool(name="p1", bufs=1) as p1, \
                 tc.tile_pool(name="p1x", bufs=2) as p1x:
                lnin = p1.tile([128, L], F32, tag="lnin")

                # conv1x1: psum[f] accumulates over 6 channel chunks
                with tc.tile_pool(name="ps_conv", bufs=1,
                                  space="PSUM") as ps_conv:
                    cps = [ps_conv.tile([D, 512], F32, tag=f"cps{f}",
                                        name=f"cps{f}")
                           for f in range(8)]
                    for c in range(6):
                        xin_c = p1x.tile([128, L], F32, tag="xin")
                        nc.sync.dma_start(xin_c[:],
                                          xin[:][c * 128:(c + 1) * 128, :])
                        for f in range(8):
                            nc.tensor.matmul(cps[f][:], convT_sb[:, c, :],
                                             xin_c[:, f * 512:(f + 1) * 512],
                                             start=(c == 0), stop=(c == 5))
                    for f in range(8):
                        nc.scalar.activation(lnin[0:D, f * 512:(f + 1) * 512],
                                             cps[f][:], AF.Identity,
                                             bias=conv_b_sb[:, 0:1])
                nc.sync.dma_start(x_out[:], lnin[0:D, :])
                lnin_bf = p1.tile([D, L], BF16, tag="lninbf")
                nc.scalar.copy(lnin_bf[:], lnin[0:D, :])

                # ---- LN1 stats
                nc.scalar.square(lnin[D:2 * D, :], lnin[0:D, :])
                mu_b = p1.tile([128, L], F32, tag="mu_b")
                rs_b = p1.tile([128, L], F32, tag="rs_b")
                st0_sb = mu_b
                st1_sb = rs_b
                with tc.tile_pool(name="ps_st", bufs=1, space="PSUM") as ps_st:
                    for hh in range(2):
                        hsl = slice(hh * 2048, (hh + 1) * 2048)
                        st0 = ps_st.tile([1, L // 2], F32, tag="st0",
                                         name="st0")
                        st1 = ps_st.tile([1, L // 2], F32, tag="st1",
                                         name="st1")
                        for f in range(4):
                            fsl = slice(hh * 2048 + f * 512,
                                        hh * 2048 + (f + 1) * 512)
                            psl = slice(f * 512, (f + 1) * 512)
                            nc.tensor.matmul(st0[:, psl], sel_sb[:, 0:1],
                                             lnin[:, fsl], start=True,
                                             stop=True)
                            nc.tensor.matmul(st1[:, psl], sel_sb[:, 1:2],
                                             lnin[:, fsl], start=True,
                                             stop=True)
                        nc.scalar.copy(st0_sb[0:1, hsl], st0[:])
                        nc.scalar.copy(st1_sb[0:1, hsl], st1[:])

                s0r = p1.tile([128, 32], F32, tag="s0r")
                s1r = p1.tile([128, 32], F32, tag="s1r")
                nc.sync.dma_start(s0r[:], st0_sb[0:1, :])
                nc.sync.dma_start(s1r[:], st1_sb[0:1, :])
                m_r = p1.tile([128, 32], F32, tag="m_r")
                nc.scalar.mul(m_r[:], s0r[:], 1.0 / D)
                msq = p1.tile([128, 32], F32, tag="msq")
                nc.scalar.square(msq[:], m_r[:])
                v_r = p1.tile([128, 32], F32, tag="v_r")
                nc.vector.scalar_tensor_tensor(v_r[:], s1r[:], 1.0 / D, msq[:],
                                               OP.mult, OP.subtract)
                sd_r = p1.tile([128, 32], F32, tag="sd_r")
                nc.scalar.activation(sd_r[:], v_r[:], AF.Sqrt, bias=eps_sb[:, 0:1])
                rs_r = p1.tile([128, 32], F32, tag="rs_r")
                nc.vector.reciprocal(rs_r[:], sd_r[:])
                nc.sync.dma_start(mu_b[0:1, :], m_r[:])
                nc.sync.dma_start(rs_b[0:1, :], rs_r[:])
                nc.gpsimd.partition_broadcast(mu_b[:], mu_b[0:1, :])
                nc.gpsimd.partition_broadcast(rs_b[:], rs_b[0:1, :])

                # ---- in_proj + LN fixup
                with tc.tile_pool(name="ps_ip", bufs=4, space="PSUM") as ps_ip, \
                     tc.tile_pool(name="fix", bufs=3) as fix:
                    for mc in range(2):
                        for f in range(8):
                            fsl = slice(f * 512, (f + 1) * 512)
                            pp = ps_ip.tile([128, 512], F32, tag="ipps")
                            nc.tensor.matmul(pp[:],
                                             Wp_sb[:, mc * 128:(mc + 1) * 128],
                                             lnin_bf[:, fsl],
                                             start=True, stop=True)
                            t1 = fix.tile([128, 512], F32, tag="t1")
                            nc.vector.scalar_tensor_tensor(
                                t1[:], mu_b[:, fsl], negq_sb[:, mc:mc + 1],
                                pp[:], OP.mult, OP.add)
                            if mc == 0:
                                r0 = f * 8
                                nc.vector.tensor_tensor(
                                    img3[:, 1 + r0:1 + r0 + 8, 1:65],
                                    t1[:].rearrange("p (r w) -> p r w", r=8),
                                    rs_b[:, fsl].rearrange("p (r w) -> p r w",
                                                           r=8),
                                    OP.mult)
                            else:
                                t2 = fix.tile([128, 512], F32, tag="t2")
                                nc.vector.tensor_tensor(t2[:], t1[:],
                                                        rs_b[:, fsl], OP.mult)
                                szt = fix.tile([128, 512], F32, tag="szt")
                                nc.scalar.activation(szt[:], t2[:],
                                                     AF.Silu,
                                                     bias=bias_z_sb[:, 0:1])
                                nc.sync.dma_start(sz_out[:][:, fsl], szt[:])

            # ---- interleaved per-chunk pipeline: dwconv -> x_proj ->
            # delta -> scans, so chunk c's scans overlap chunk c+1's prep
            dts_sb = main.tile([R, L], BF16, tag="dts")
            with tc.tile_pool(name="ps_dw", bufs=1, space="PSUM") as ps_dw, \
                 tc.tile_pool(name="ps_xp", bufs=1, space="PSUM") as ps_xp, \
                 tc.tile_pool(name="ps_dt", bufs=1, space="PSUM") as ps_dt, \
                 tc.tile_pool(name="ps_bb", bufs=1, space="PSUM") as ps_bb, \
                 tc.tile_pool(name="nl", bufs=3) as nl, \
                 tc.tile_pool(name="yp", bufs=2) as yp:
            
                for c in range(NCH):
                    csl = slice(c * LC, (c + 1) * LC)
                    for f in range(2 * c, 2 * c + 2):
                        fsl = slice(f * 512, (f + 1) * 512)
                        r0 = f * 8
                        dps = ps_dw.tile([128, 512], F32, tag="dwps")
                        for t in range(9):
                            di_, dj = t // 3, t % 3
                            nc.tensor.matmul(
                                dps[:], dwdiag_sb[:, t, :],
                                img3[:, r0 + di_:r0 + di_ + 8, dj:dj + 64],
                                start=(t == 0), stop=(t == 8))
                        nc.scalar.activation(xc_sb[:, fsl], dps[:], AF.Silu,
                                             bias=bias_dw_sb[:, 0:1])
                        xps = ps_xp.tile([R + 2 * N, 512], F32, tag="xpps")
                        nc.tensor.matmul(xps[:], xprojT_sb[:], xc_sb[:, fsl],
                                         start=True, stop=True)
                        nc.scalar.copy(xdbl_bf[:, fsl], xps[:])
                        nc.scalar.copy(dts_sb[:, fsl], xps[0:R, :])
                        dtps = ps_dt.tile([Di, 512], F32, tag="dtps")
                        nc.tensor.matmul(dtps[:], dtT_sb[:], dts_sb[:, fsl],
                                         start=True, stop=True)
                        nc.scalar.activation(delta_sb[:, fsl], dtps[:],
                                             AF.Sigmoid,
                                             bias=dtb_sb[:, 0:1], scale=-1.0)
                    nc.scalar.activation(delta_sb[:, csl], delta_sb[:, csl],
                                         AF.Ln)
                    nc.vector.scalar_tensor_tensor(du_sb[:, csl],
                                                   delta_sb[:, csl],
                                                   -1.0, xc_sb[:, csl],
                                                   OP.mult, OP.mult)
                    y_acc = yp.tile([Di, LC], F32, tag="yacc")
                    for n in range(N):
                        bb = ps_bb.tile([128, LC], F32, tag="bb")
                        for j in range(LC // 512):
                            nc.tensor.matmul(
                                bb[:, j * 512:(j + 1) * 512],
                                bsel_sb[:, n * 128:(n + 1) * 128],
                                xdbl_bf[:, c * LC + j * 512:
                                        c * LC + (j + 1) * 512],
                                start=True, stop=True)
                        cb = ps_bb.tile([128, LC], F32, tag="cb")
                        for j in range(LC // 512):
                            nc.tensor.matmul(
                                cb[:, j * 512:(j + 1) * 512],
                                csel_sb[:, n * 128:(n + 1) * 128],
                                xdbl_bf[:, c * LC + j * 512:
                                        c * LC + (j + 1) * 512],
                                start=True, stop=True)
                        da = nl.tile([Di, LC], F32, tag="da")
                        nc.scalar.activation(da[:], delta_sb[:, csl], AF.Exp,
                                             scale=A_sb[:, n:n + 1])
                        dbu = nl.tile([Di, LC], F32, tag="dbu")
                        nc.vector.tensor_tensor(dbu[:], du_sb[:, csl], bb[:],
                                                OP.mult)
                        h = nl.tile([Di, LC], F32, tag="h")
                        nc.vector.tensor_tensor_scan(
                            h[:], da[:], dbu[:],
                            0.0 if c == 0 else carry_sb[:, n:n + 1],
                            OP.mult, OP.add)
                        if c < NCH - 1:
                            nc.vector.tensor_copy(carry_sb[:, n:n + 1],
                                                  h[:, LC - 1:LC])
                        if n == 0:
                            nc.vector.tensor_tensor(y_acc[:], h[:], cb[:],
                                                    OP.mult)
                        else:
                            tmp = nl.tile([Di, LC], F32, tag="tmp")
                            nc.vector.tensor_tensor(tmp[:], h[:], cb[:],
                                                    OP.mult)
                            nc.gpsimd.tensor_tensor(y_acc[:], y_acc[:],
                                                    tmp[:], OP.add)
                    y_f = yp.tile([Di, LC], F32, tag="yout")
                    nc.vector.scalar_tensor_tensor(y_f[:], xc_sb[:, csl],
                                                   Ds_sb[:, 0:1], y_acc[:],
                                                   OP.mult, OP.add)
                    nc.sync.dma_start(y_out[:][:, csl], y_f[:])

    nc.compile()
    return nc


def _finish(nc):
    return nc


# ---------------------------------------------------------------- launch 2

def build_launch2():
    nc = bacc.Bacc("TRN2", target_bir_lowering=False, debug=False,
                   num_devices=8)

    def inp(name, shape):
        return nc.dram_tensor(name, shape, F32, kind="ExternalInput")

    y_in = nc.dram_tensor("y_in", [Di, T2], BF16, kind="ExternalInput")
    sz_in = nc.dram_tensor("sz_in", [Di, T2], BF16, kind="ExternalInput")
    x_in = inp("x_in", [D, T2])
    ones128 = inp("ones128", [128, 1])
    onorm_g = inp("onorm_g", [Di, 1])
    onorm_b = inp("onorm_b", [Di, 1])
    oproj = inp("oproj", [Di, D])
    fc1p = inp("fc1p", [D, 2 * Di])
    bias1 = inp("bias1", [128, 2])
    fc2w = inp("fc2w", [2 * Di, D])
    fc2b = inp("fc2b", [D, 1])
    out = nc.dram_tensor("out", [D, T2], F32, kind="ExternalOutput")

    with tile.TileContext(nc) as tc, ExitStack() as ctx:
        po = ctx.enter_context(tc.tile_pool(name="main", bufs=1))
        ps = ctx.enter_context(tc.tile_pool(name="psum", bufs=1, space="PSUM"))

        y_sb = po.tile([Di, T2], BF16, tag="y")
        nc.sync.dma_start(y_sb[:], y_in[:])
        sz_sb = po.tile([Di, T2], BF16, tag="sz")
        nc.sync.dma_start(sz_sb[:], sz_in[:])
        x_sb = po.tile([D, T2], F32, tag="x")
        nc.sync.dma_start(x_sb[:], x_in[:])
        ones_sb = po.tile([128, 1], F32, tag="ones")
        nc.sync.dma_start(ones_sb[:], ones128[:])
        ones_bf = po.tile([128, 1], BF16, tag="onesbf")
        nc.vector.memset(ones_bf[:], 1.0)
        og_sb = po.tile([Di, 1], F32, tag="og")
        nc.sync.dma_start(og_sb[:], onorm_g[:])
        ob_sb = po.tile([Di, 1], F32, tag="ob")
        nc.sync.dma_start(ob_sb[:], onorm_b[:])
        op_sb = po.tile([Di, D], F32, tag="oproj")
        nc.sync.dma_start(op_sb[:], oproj[:])
        fc1_sb = po.tile([D, 2 * Di], F32, tag="fc1")
        nc.sync.dma_start(fc1_sb[:], fc1p[:])
        b1_sb = po.tile([128, 2], F32, tag="b1")
        nc.sync.dma_start(b1_sb[:], bias1[:])
        fc2_sb = po.tile([128, 2, D], F32, tag="fc2")
        nc.sync.dma_start(fc2_sb[:], fc2w[:].rearrange("(c p) m -> p c m", p=128))
        fc2b_sb = po.tile([D, 1], F32, tag="fc2b")
        nc.sync.dma_start(fc2b_sb[:], fc2b[:])
        eps_sb = po.tile([128, 1], F32, tag="eps")
        nc.vector.memset(eps_sb[:], EPS)

        def pln(src, parts, tag, dt=F32, ones_t=None):
            """LayerNorm stats over the partition dim of src [parts, T2];
            returns broadcast (mu_b, rs_b) [parts, T2] tiles."""
            ones_t = ones_sb if ones_t is None else ones_t
            sq = po.tile([parts, T2], dt, tag=tag + "sq")
            nc.scalar.square(sq[:], src)
            st0_sb = po.tile([1, T2], F32, tag=tag + "st0sb")
            st1_sb = po.tile([1, T2], F32, tag=tag + "st1sb")
            with tc.tile_pool(name=tag + "ps_st", bufs=1,
                              space="PSUM") as ps_st:
                st0 = ps_st.tile([1, T2], F32, tag="st0")
                st1 = ps_st.tile([1, T2], F32, tag="st1")
                for f in range(T2 // 512):
                    fsl = slice(f * 512, (f + 1) * 512)
                    nc.tensor.matmul(st0[:, fsl], ones_t[0:parts, :],
                                     src[:, fsl], start=True, stop=True)
                    nc.tensor.matmul(st1[:, fsl], ones_t[0:parts, :],
                                     sq[:, fsl], start=True, stop=True)
                nc.scalar.copy(st0_sb[:], st0[:])
                nc.scalar.copy(st1_sb[:], st1[:])
            s0r = po.tile([128, T2 // 128], F32, tag=tag + "s0r")
            s1r = po.tile([128, T2 // 128], F32, tag=tag + "s1r")
            nc.sync.dma_start(s0r[:], st0_sb[:])
            nc.sync.dma_start(s1r[:], st1_sb[:])
            m_r = po.tile([128, T2 // 128], F32, tag=tag + "m")
            nc.scalar.mul(m_r[:], s0r[:], 1.0 / parts)
            msq = po.tile([128, T2 // 128], F32, tag=tag + "msq")
            nc.scalar.square(msq[:], m_r[:])
            v_r = po.tile([128, T2 // 128], F32, tag=tag + "v")
            nc.vector.scalar_tensor_tensor(v_r[:], s1r[:], 1.0 / parts,
                                           msq[:], OP.mult, OP.subtract)
            sd_r = po.tile([128, T2 // 128], F32, tag=tag + "sd")
            nc.scalar.activation(sd_r[:], v_r[:], AF.Sqrt, bias=eps_sb[:parts if False else 128, 0:1])
            rs_r = po.tile([128, T2 // 128], F32, tag=tag + "rs")
            nc.vector.reciprocal(rs_r[:], sd_r[:])
            mu1 = po.tile([1, T2], F32, tag=tag + "mu1")
            rs1 = po.tile([1, T2], F32, tag=tag + "rs1")
            nc.sync.dma_start(mu1[:], m_r[:])
            nc.sync.dma_start(rs1[:], rs_r[:])
            mu_b = po.tile([parts, T2], F32, tag=tag + "mub")
            rs_b = po.tile([parts, T2], F32, tag=tag + "rsb")
            nc.gpsimd.partition_broadcast(mu_b[:], mu1[:])
            nc.gpsimd.partition_broadcast(rs_b[:], rs1[:])
            return mu_b, rs_b

        # out_norm (over Di) + gate
        mu_b, rs_b = pln(y_sb[:], Di, "a", dt=BF16, ones_t=ones_bf)
        t1 = po.tile([Di, T2], F32, tag="t1")
        nc.vector.tensor_tensor(t1[:], y_sb[:], mu_b[:], OP.subtract)
        t2 = po.tile([Di, T2], F32, tag="t2")
        nc.vector.tensor_tensor(t2[:], t1[:], rs_b[:], OP.mult)
        t3 = po.tile([Di, T2], F32, tag="t3")
        nc.vector.tensor_scalar(t3[:], t2[:], og_sb[:, 0:1], ob_sb[:, 0:1],
                                OP.mult, OP.add)
        yg = po.tile([Di, T2], F32, tag="yg")
        nc.vector.tensor_tensor(yg[:], t3[:], sz_sb[:], OP.mult)

        # out_proj + residual ;  "mm" psum tag shared/serialized
        x2 = po.tile([D, T2], F32, tag="x2")
        opps = ps.tile([128, T2], F32, tag="mm")
        for f in range(T2 // 512):
            fsl = slice(f * 512, (f + 1) * 512)
            nc.tensor.matmul(opps[0:D, fsl], op_sb[:], yg[:, fsl],
                             start=True, stop=True)
        nc.vector.tensor_tensor(x2[:], opps[0:D, :], x_sb[:], OP.add)

        # LN2 (over D) -> fc1 -> gelu -> fc2 -> + residual
        mu2, rs2 = pln(x2[:], D, "b")
        h1 = po.tile([D, T2], F32, tag="h1")
        nc.vector.tensor_tensor(h1[:], x2[:], mu2[:], OP.subtract)
        hn = po.tile([D, T2], F32, tag="hn")
        nc.vector.tensor_tensor(hn[:], h1[:], rs2[:], OP.mult)

        g1 = po.tile([128, 2, T2], F32, tag="g1")
        for mc in range(2):
            fp = ps.tile([128, T2], F32, tag="mm")
            for f in range(T2 // 512):
                fsl = slice(f * 512, (f + 1) * 512)
                nc.tensor.matmul(fp[:, fsl],
                                 fc1_sb[:, mc * 128:(mc + 1) * 128],
                                 hn[:, fsl], start=True, stop=True)
            nc.scalar.activation(g1[:, mc, :], fp[:],
                                 AF.Gelu_apprx_tanh, bias=b1_sb[:, mc:mc + 1])
        f2 = ps.tile([128, T2], F32, tag="mm")
        for f in range(T2 // 512):
            fsl = slice(f * 512, (f + 1) * 512)
            for mc in range(2):
                nc.tensor.matmul(f2[0:D, fsl], fc2_sb[:, mc, :],
                                 g1[:, mc, fsl],
                                 start=(mc == 0), stop=(mc == 1))
        o_sb = po.tile([D, T2], F32, tag="o")
        nc.vector.scalar_tensor_tensor(o_sb[:], f2[0:D, :], fc2b_sb[:, 0:1],
                                       x2[:], OP.add, OP.add)
        nc.sync.dma_start(out[:], o_sb[:])

    nc.compile()
    return nc


# ---------------------------------------------------------------- host side

_CACHE = {}


def _get_programs():
    if "nc1" not in _CACHE:
        _CACHE["nc1"] = build_launch1()
        _CACHE["nc2"] = build_launch2()
    return _CACHE["nc1"], _CACHE["nc2"]


def _prep_inmaps(inputs):
    f32 = lambda a: np.ascontiguousarray(np.asarray(a), dtype=np.float32)
    conv_w = f32(inputs["conv_w"])
    conv_b = f32(inputs["conv_b"])
    ln1_g, ln1_b = f32(inputs["ln1_g"]), f32(inputs["ln1_b"])
    in_proj_w = f32(inputs["in_proj_w"])
    dw_w_all = f32(inputs["conv_dw_w"])[:, 0]
    dw_b = f32(inputs["conv_dw_b"])
    x_proj_w = f32(inputs["x_proj_w"])
    dt_proj_w = f32(inputs["dt_proj_w"])
    dt_proj_b = f32(inputs["dt_proj_b"])
    A = np.exp(f32(inputs["A_logs"])).reshape(K, Di, N).astype(np.float32)
    Ds = f32(inputs["Ds"]).reshape(K, Di)

    Wp = (ln1_g[:, None] * in_proj_w).astype(np.float32)        # [64, 256]
    Wp_bf = Wp.astype(ml_dtypes.bfloat16)
    q = Wp.sum(0)
    bias_full = (ln1_b @ in_proj_w).astype(np.float32)          # [256]
    negq = np.ascontiguousarray(np.stack([-q[:Di], -q[Di:]], 1), np.float32)
    sel = np.zeros((128, 2), np.float32)
    sel[:D, 0] = 1.0
    sel[D:, 1] = 1.0
    ones1 = np.ones((1, 128), np.float32)

    Ps = _perms()
    x123 = [np.concatenate([f32(inputs["x1"])[b], f32(inputs["x2"])[b],
                            f32(inputs["x3"])[b]], 0).reshape(3 * C, L)
            for b in range(B)]

    bsel_np = np.zeros((R + 2 * N, N * 128), ml_dtypes.bfloat16)
    csel_np = np.zeros((R + 2 * N, N * 128), ml_dtypes.bfloat16)
    for n in range(N):
        bsel_np[R + n, n * 128:(n + 1) * 128] = 1.0
        csel_np[R + N + n, n * 128:(n + 1) * 128] = 1.0
    shared = {
        "bsel": bsel_np, "csel": csel_np,
        "convT": np.ascontiguousarray(conv_w.T),
        "conv_b": conv_b.reshape(D, 1).copy(),
        "sel": sel, "ones1": ones1, "Wp": Wp_bf, "negq": negq,
        "bias_z": bias_full[Di:].reshape(Di, 1).copy(),
    }
    in_maps = []
    for core in range(8):
        b, k = core // 4, core % 4
        dw_w = _permute_kernel(dw_w_all, k)
        wsum = dw_w.sum((1, 2))
        dwdiag = np.zeros((9, 128, 128), ml_dtypes.bfloat16)
        for t in range(9):
            np.fill_diagonal(dwdiag[t], dw_w[:, t // 3, t % 3])
        in_maps.append({
            **shared,
            "xin": np.ascontiguousarray(x123[b][:, Ps[k]]),
            "dwdiag": dwdiag,
            "bias_dw": (dw_b + bias_full[:Di] * wsum).reshape(Di, 1)
                        .astype(np.float32),
            "xprojT": np.ascontiguousarray(x_proj_w[k].T.astype(ml_dtypes.bfloat16)),
            "dtT": np.ascontiguousarray(dt_proj_w[k].T.astype(ml_dtypes.bfloat16)),
            "dtb": (-dt_proj_b[k]).reshape(Di, 1).copy(),
            "A_in": np.ascontiguousarray(A[k]),
            "Ds_in": Ds[k].reshape(Di, 1).copy(),
        })
    return in_maps, Ps


def _prep_inmaps2(inputs, y_merged, sz_full, x_full):
    f32 = lambda a: np.ascontiguousarray(np.asarray(a), dtype=np.float32)
    ln2_g, ln2_b = f32(inputs["ln2_g"]), f32(inputs["ln2_b"])
    fc1_w, fc1_b = f32(inputs["fc1_w"]), f32(inputs["fc1_b"])
    fc1p = (ln2_g[:, None] * fc1_w).astype(np.float32)
    bias1 = (ln2_b @ fc1_w + fc1_b).astype(np.float32)
    shared = {
        "ones128": np.ones((128, 1), np.float32),
        "onorm_g": f32(inputs["out_norm_g"]).reshape(Di, 1).copy(),
        "onorm_b": f32(inputs["out_norm_b"]).reshape(Di, 1).copy(),
        "oproj": f32(inputs["out_proj_w"]),
        "fc1p": fc1p,
        "bias1": np.ascontiguousarray(np.stack([bias1[:128], bias1[128:]], 1),
                                      np.float32),
        "fc2w": f32(inputs["fc2_w"]),
        "fc2b": f32(inputs["fc2_b"]).reshape(D, 1).copy(),
    }
    in_maps = []
    for core in range(8):
        b, sl = core // 4, slice((core % 4) * T2, (core % 4 + 1) * T2)
        in_maps.append({
            **shared,
            "y_in": np.ascontiguousarray(
                y_merged[b][:, sl].astype(ml_dtypes.bfloat16)),
            "sz_in": np.ascontiguousarray(
                sz_full[b][:, sl].astype(ml_dtypes.bfloat16)),
            "x_in": np.ascontiguousarray(x_full[b][:, sl]),
        })
    return in_maps


def kernel(**inputs):
    nc1, nc2 = _get_programs()
    in_maps, Ps = _prep_inmaps(inputs)
    res1 = run_bass_kernel_spmd(nc1, in_maps, list(range(8))).results

    y_merged = np.zeros((B, Di, L), np.float32)
    sz_full = [None] * B
    x_full = [None] * B
    for core in range(8):
        b, k = core // 4, core % 4
        y_merged[b][:, Ps[k]] += res1[core]["y_out"]
        if k == 0:
            sz_full[b] = res1[core]["sz_out"]
            x_full[b] = res1[core]["x_out"]

    in_maps2 = _prep_inmaps2(inputs, y_merged, sz_full, x_full)
    res2 = run_bass_kernel_spmd(nc2, in_maps2, list(range(8))).results

    out = np.zeros((B, D, L), np.float32)
    for core in range(8):
        b, sl = core // 4, slice((core % 4) * T2, (core % 4 + 1) * T2)
        out[b][:, sl] = res2[core]["out"]
    return out.reshape(B, D, H, W)


# revision 37
# speedup vs baseline: 1.2087x; 1.0454x over previous
"""Trainium2 Bass kernel for nn_Decoder_17489106830107 (VMamba VSSBlock decoder).

Sharding: one (batch, scan-direction) pair per core (B=2 x K=4 = 8 cores).
The host pre-permutes each core's inputs into that core's scan coordinate
order (transpose / 180-rotation of the image), so all 8 cores run ONE
identical SPMD program for launch 1 (conv1x1 -> LN -> in_proj -> depthwise
conv -> x_proj/dt -> 16 hardware linear-recurrence scans on the DVE).
The host then scatter-adds the 4 directional outputs per batch and an
8-way token-parallel launch 2 does the merge epilogue (out_norm, gating,
out_proj, MLP).
"""
import numpy as np
from contextlib import ExitStack

import concourse.bacc as bacc
import concourse.bass as bass
import concourse.mybir as mybir
import concourse.tile as tile
from concourse.bass_utils import run_bass_kernel_spmd
import ml_dtypes

F32 = mybir.dt.float32
F32R = mybir.dt.float32r
BF16 = mybir.dt.bfloat16
AF = mybir.ActivationFunctionType
OP = mybir.AluOpType

B, C, H, W = 2, 256, 64, 64
D = 64
Di = 128
N = 16
R = 4
K = 4
L = H * W          # 4096
LC = 1024          # scan chunk
NCH = L // LC
EPS = 1e-5
T2 = 1024          # launch-2 token slice per core


# ---------------------------------------------------------------- host prep

def _perms():
    ar = np.arange(L)
    p1 = (ar % 64) * 64 + ar // 64
    return [ar, p1, ar[::-1].copy(), p1[::-1].copy()]


def _permute_kernel(w, k):
    if k == 0:
        return w
    if k == 1:
        return w.transpose(0, 2, 1)
    if k == 2:
        return w[:, ::-1, ::-1]
    return w.transpose(0, 2, 1)[:, ::-1, ::-1]


# ---------------------------------------------------------------- launch 1

def _r(ap):
    return ap.bitcast(F32R)


def build_launch1():
    nc = bacc.Bacc("TRN2", target_bir_lowering=False, debug=False,
                   num_devices=8)

    def inp(name, shape):
        return nc.dram_tensor(name, shape, F32, kind="ExternalInput")

    xin = inp("xin", [3 * C, L])
    convT = inp("convT", [3 * C, D])
    conv_b = inp("conv_b", [D, 1])
    sel = inp("sel", [128, 2])
    ones1 = inp("ones1", [1, 128])
    Wp = nc.dram_tensor("Wp", [D, 2 * Di], BF16,
                        kind="ExternalInput")
    negq = inp("negq", [128, 2])
    bias_z = inp("bias_z", [128, 1])
    dwdiag = nc.dram_tensor("dwdiag", [9, 128, 128], BF16,
                            kind="ExternalInput")
    bias_dw = inp("bias_dw", [128, 1])
    xprojT = nc.dram_tensor("xprojT", [Di, R + 2 * N], BF16,
                            kind="ExternalInput")
    dtT = nc.dram_tensor("dtT", [R, Di], BF16,
                         kind="ExternalInput")
    dtb = inp("dtb", [Di, 1])
    A_in = inp("A_in", [Di, N])
    bsel = nc.dram_tensor("bsel", [R + 2 * N, N * 128], BF16,
                          kind="ExternalInput")
    csel = nc.dram_tensor("csel", [R + 2 * N, N * 128], BF16,
                          kind="ExternalInput")
    Ds_in = inp("Ds_in", [Di, 1])

    y_out = nc.dram_tensor("y_out", [Di, L], F32, kind="ExternalOutput")
    sz_out = nc.dram_tensor("sz_out", [Di, L], F32, kind="ExternalOutput")
    x_out = nc.dram_tensor("x_out", [D, L], F32, kind="ExternalOutput")

    with tile.TileContext(nc) as tc, ExitStack() as ctx:
        cpool = ctx.enter_context(tc.tile_pool(name="consts", bufs=1))
        main = ctx.enter_context(tc.tile_pool(name="main", bufs=1))

        # ---- const loads
        convT_sb = cpool.tile([128, 6, D], F32, tag="convT")
        nc.sync.dma_start(convT_sb[:], convT[:].rearrange("(c p) m -> p c m", p=128))
        conv_b_sb = cpool.tile([D, 1], F32, tag="convb")
        nc.sync.dma_start(conv_b_sb[:], conv_b[:])
        sel_sb = cpool.tile([128, 2], F32, tag="sel")
        nc.sync.dma_start(sel_sb[:], sel[:])
        ones1_sb = cpool.tile([1, 128], F32, tag="ones1")
        nc.sync.dma_start(ones1_sb[:], ones1[:])
        Wp_sb = cpool.tile([D, 2 * Di], BF16, tag="Wp")
        nc.sync.dma_start(Wp_sb[:], Wp[:])
        negq_sb = cpool.tile([128, 2], F32, tag="negq")
        nc.sync.dma_start(negq_sb[:], negq[:])
        bias_z_sb = cpool.tile([128, 1], F32, tag="biasz")
        nc.sync.dma_start(bias_z_sb[:], bias_z[:])
        dwdiag_sb = cpool.tile([128, 9, 128], BF16, tag="dwdiag")
        nc.sync.dma_start(dwdiag_sb[:], dwdiag[:].rearrange("t p f -> p t f"))
        bias_dw_sb = cpool.tile([128, 1], F32, tag="biasdw")
        nc.sync.dma_start(bias_dw_sb[:], bias_dw[:])
        xprojT_sb = cpool.tile([Di, R + 2 * N], BF16, tag="xprojT")
        nc.sync.dma_start(xprojT_sb[:], xprojT[:])
        dtT_sb = cpool.tile([R, Di], BF16, tag="dtT")
        nc.sync.dma_start(dtT_sb[:], dtT[:])
        dtb_sb = cpool.tile([Di, 1], F32, tag="dtb")
        nc.sync.dma_start(dtb_sb[:], dtb[:])
        A_sb = cpool.tile([Di, N], F32, tag="A")
        nc.sync.dma_start(A_sb[:], A_in[:])
        Ds_sb = cpool.tile([Di, 1], F32, tag="Ds")
        nc.sync.dma_start(Ds_sb[:], Ds_in[:])
        eps_sb = cpool.tile([128, 1], F32, tag="eps")
        nc.vector.memset(eps_sb[:], EPS)
        bsel_sb = cpool.tile([R + 2 * N, N * 128], BF16, tag="bsel")
        nc.sync.dma_start(bsel_sb[:], bsel[:])
        csel_sb = cpool.tile([R + 2 * N, N * 128], BF16, tag="csel")
        nc.sync.dma_start(csel_sb[:], csel[:])

        # ---- persistent activations
        xc_sb = main.tile([Di, L], BF16, tag="xc")
        xdbl_bf = main.tile([R + 2 * N, L], BF16, tag="xdblbf")
        delta_sb = main.tile([Di, L], F32, tag="delta")
        du_sb = main.tile([Di, L], F32, tag="du")
        carry_sb = main.tile([Di, N], F32, tag="carry")

        with tc.tile_pool(name="imgp", bufs=1) as imgp:
            img = imgp.tile([Di, 66 * 66], BF16, tag="img")
            nc.gpsimd.memset(img[:], 0.0)
            img3 = img[:].rearrange("p (h w) -> p h w", h=66)

            with tc.tile_pool(name="p1", bufs=1) as p1, \
                 tc.tile_pool(name="p1x", bufs=2) as p1x:
                lnin = p1.tile([128, L], F32, tag="lnin")

                # conv1x1: psum[f] accumulates over 6 channel chunks
                with tc.tile_pool(name="ps_conv", bufs=1,
                                  space="PSUM") as ps_conv:
                    cps = [ps_conv.tile([D, 512], F32, tag=f"cps{f}",
                                        name=f"cps{f}")
                           for f in range(8)]
                    for c in range(6):
                        xin_c = p1x.tile([128, L], F32, tag="xin")
                        nc.sync.dma_start(xin_c[:],
                                          xin[:][c * 128:(c + 1) * 128, :])
                        for f in range(8):
                            nc.tensor.matmul(cps[f][:], convT_sb[:, c, :],
                                             xin_c[:, f * 512:(f + 1) * 512],
                                             start=(c == 0), stop=(c == 5))
                    for f in range(8):
                        nc.scalar.activation(lnin[0:D, f * 512:(f + 1) * 512],
                                             cps[f][:], AF.Identity,
                                             bias=conv_b_sb[:, 0:1])
                nc.sync.dma_start(x_out[:], lnin[0:D, :])
                lnin_bf = p1.tile([D, L], BF16, tag="lninbf")
                nc.scalar.copy(lnin_bf[:], lnin[0:D, :])

                # ---- LN1 stats, fully per-half so downstream starts early
                mu_b = p1.tile([128, L], F32, tag="mu_b")
                rs_b = p1.tile([128, L], F32, tag="rs_b")
                with tc.tile_pool(name="ps_st", bufs=1, space="PSUM") as ps_st:
                    for hh in range(2):
                        hsl = slice(hh * 2048, (hh + 1) * 2048)
                        nc.scalar.square(lnin[D:2 * D, hsl], lnin[0:D, hsl])
                        st0 = ps_st.tile([1, L // 2], F32, tag="st0",
                                         name="st0")
                        st1 = ps_st.tile([1, L // 2], F32, tag="st1",
                                         name="st1")
                        for f in range(4):
                            fsl = slice(hh * 2048 + f * 512,
                                        hh * 2048 + (f + 1) * 512)
                            psl = slice(f * 512, (f + 1) * 512)
                            nc.tensor.matmul(st0[:, psl], sel_sb[:, 0:1],
                                             lnin[:, fsl], start=True,
                                             stop=True)
                            nc.tensor.matmul(st1[:, psl], sel_sb[:, 1:2],
                                             lnin[:, fsl], start=True,
                                             stop=True)
                        nc.scalar.copy(mu_b[0:1, hsl], st0[:])
                        nc.scalar.copy(rs_b[0:1, hsl], st1[:])
                        s0r = p1.tile([128, 16], F32, tag="s0r", bufs=2)
                        s1r = p1.tile([128, 16], F32, tag="s1r", bufs=2)
                        nc.sync.dma_start(s0r[:], mu_b[0:1, hsl])
                        nc.sync.dma_start(s1r[:], rs_b[0:1, hsl])
                        m_r = p1.tile([128, 16], F32, tag="m_r", bufs=2)
                        nc.scalar.mul(m_r[:], s0r[:], 1.0 / D)
                        msq = p1.tile([128, 16], F32, tag="msq", bufs=2)
                        nc.scalar.square(msq[:], m_r[:])
                        v_r = p1.tile([128, 16], F32, tag="v_r", bufs=2)
                        nc.vector.scalar_tensor_tensor(v_r[:], s1r[:], 1.0 / D,
                                                       msq[:], OP.mult,
                                                       OP.subtract)
                        sd_r = p1.tile([128, 16], F32, tag="sd_r", bufs=2)
                        nc.scalar.activation(sd_r[:], v_r[:], AF.Sqrt,
                                             bias=eps_sb[:, 0:1])
                        rs_r = p1.tile([128, 16], F32, tag="rs_r", bufs=2)
                        nc.vector.reciprocal(rs_r[:], sd_r[:])
                        nc.sync.dma_start(mu_b[0:1, hsl], m_r[:])
                        nc.sync.dma_start(rs_b[0:1, hsl], rs_r[:])
                        nc.gpsimd.partition_broadcast(mu_b[:, hsl],
                                                      mu_b[0:1, hsl])
                        nc.gpsimd.partition_broadcast(rs_b[:, hsl],
                                                      rs_b[0:1, hsl])

                # ---- in_proj + LN fixup
                with tc.tile_pool(name="ps_ip", bufs=4, space="PSUM") as ps_ip, \
                     tc.tile_pool(name="fix", bufs=3) as fix:
                    for mc in range(2):
                        for f in range(8):
                            fsl = slice(f * 512, (f + 1) * 512)
                            pp = ps_ip.tile([128, 512], F32, tag="ipps")
                            nc.tensor.matmul(pp[:],
                                             Wp_sb[:, mc * 128:(mc + 1) * 128],
                                             lnin_bf[:, fsl],
                                             start=True, stop=True)
                            t1 = fix.tile([128, 512], F32, tag="t1")
                            nc.vector.scalar_tensor_tensor(
                                t1[:], mu_b[:, fsl], negq_sb[:, mc:mc + 1],
                                pp[:], OP.mult, OP.add)
                            if mc == 0:
                                r0 = f * 8
                                nc.vector.tensor_tensor(
                                    img3[:, 1 + r0:1 + r0 + 8, 1:65],
                                    t1[:].rearrange("p (r w) -> p r w", r=8),
                                    rs_b[:, fsl].rearrange("p (r w) -> p r w",
                                                           r=8),
                                    OP.mult)
                            else:
                                t2 = fix.tile([128, 512], F32, tag="t2")
                                nc.vector.tensor_tensor(t2[:], t1[:],
                                                        rs_b[:, fsl], OP.mult)
                                szt = fix.tile([128, 512], F32, tag="szt")
                                nc.scalar.activation(szt[:], t2[:],
                                                     AF.Silu,
                                                     bias=bias_z_sb[:, 0:1])
                                nc.sync.dma_start(sz_out[:][:, fsl], szt[:])

            # ---- interleaved per-chunk pipeline: dwconv -> x_proj ->
            # delta -> scans, so chunk c's scans overlap chunk c+1's prep
            dts_sb = main.tile([R, L], BF16, tag="dts")
            with tc.tile_pool(name="ps_dw", bufs=1, space="PSUM") as ps_dw, \
                 tc.tile_pool(name="ps_xp", bufs=1, space="PSUM") as ps_xp, \
                 tc.tile_pool(name="ps_dt", bufs=1, space="PSUM") as ps_dt, \
                 tc.tile_pool(name="ps_bb", bufs=1, space="PSUM") as ps_bb, \
                 tc.tile_pool(name="nl", bufs=3) as nl, \
                 tc.tile_pool(name="yp", bufs=2) as yp:
            
                for c in range(NCH):
                    csl = slice(c * LC, (c + 1) * LC)
                    for f in range(2 * c, 2 * c + 2):
                        fsl = slice(f * 512, (f + 1) * 512)
                        r0 = f * 8
                        dps = ps_dw.tile([128, 512], F32, tag="dwps")
                        for t in range(9):
                            di_, dj = t // 3, t % 3
                            nc.tensor.matmul(
                                dps[:], dwdiag_sb[:, t, :],
                                img3[:, r0 + di_:r0 + di_ + 8, dj:dj + 64],
                                start=(t == 0), stop=(t == 8))
                        nc.scalar.activation(xc_sb[:, fsl], dps[:], AF.Silu,
                                             bias=bias_dw_sb[:, 0:1])
                        xps = ps_xp.tile([R + 2 * N, 512], F32, tag="xpps")
                        nc.tensor.matmul(xps[:], xprojT_sb[:], xc_sb[:, fsl],
                                         start=True, stop=True)
                        nc.scalar.copy(xdbl_bf[:, fsl], xps[:])
                        nc.scalar.copy(dts_sb[:, fsl], xps[0:R, :])
                        dtps = ps_dt.tile([Di, 512], F32, tag="dtps")
                        nc.tensor.matmul(dtps[:], dtT_sb[:], dts_sb[:, fsl],
                                         start=True, stop=True)
                        nc.scalar.activation(delta_sb[:, fsl], dtps[:],
                                             AF.Sigmoid,
                                             bias=dtb_sb[:, 0:1], scale=-1.0)
                    nc.scalar.activation(delta_sb[:, csl], delta_sb[:, csl],
                                         AF.Ln)
                    nc.vector.scalar_tensor_tensor(du_sb[:, csl],
                                                   delta_sb[:, csl],
                                                   -1.0, xc_sb[:, csl],
                                                   OP.mult, OP.mult)
                    y_acc = yp.tile([Di, LC], F32, tag="yacc")
                    for n in range(N):
                        bb = ps_bb.tile([128, LC], F32, tag="bb")
                        for j in range(LC // 512):
                            nc.tensor.matmul(
                                bb[:, j * 512:(j + 1) * 512],
                                bsel_sb[:, n * 128:(n + 1) * 128],
                                xdbl_bf[:, c * LC + j * 512:
                                        c * LC + (j + 1) * 512],
                                start=True, stop=True)
                        cb = ps_bb.tile([128, LC], F32, tag="cb")
                        for j in range(LC // 512):
                            nc.tensor.matmul(
                                cb[:, j * 512:(j + 1) * 512],
                                csel_sb[:, n * 128:(n + 1) * 128],
                                xdbl_bf[:, c * LC + j * 512:
                                        c * LC + (j + 1) * 512],
                                start=True, stop=True)
                        da = nl.tile([Di, LC], F32, tag="da")
                        nc.scalar.activation(da[:], delta_sb[:, csl], AF.Exp,
                                             scale=A_sb[:, n:n + 1])
                        dbu = nl.tile([Di, LC], F32, tag="dbu")
                        nc.vector.tensor_tensor(dbu[:], du_sb[:, csl], bb[:],
                                                OP.mult)
                        h = nl.tile([Di, LC], F32, tag="h")
                        nc.vector.tensor_tensor_scan(
                            h[:], da[:], dbu[:],
                            0.0 if c == 0 else carry_sb[:, n:n + 1],
                            OP.mult, OP.add)
                        if c < NCH - 1:
                            nc.scalar.copy(carry_sb[:, n:n + 1],
                                           h[:, LC - 1:LC])
                        if n == 0:
                            nc.vector.tensor_tensor(y_acc[:], h[:], cb[:],
                                                    OP.mult)
                        else:
                            tmp = nl.tile([Di, LC], F32, tag="tmp")
                            nc.vector.tensor_tensor(tmp[:], h[:], cb[:],
                                                    OP.mult)
                            nc.gpsimd.tensor_tensor(y_acc[:], y_acc[:],
                                                    tmp[:], OP.add)
                    y_f = yp.tile([Di, LC], F32, tag="yout")
                    nc.vector.scalar_tensor_tensor(y_f[:], xc_sb[:, csl],
                                                   Ds_sb[:, 0:1], y_acc[:],
                                                   OP.mult, OP.add)
                    nc.sync.dma_start(y_out[:][:, csl], y_f[:])

    nc.compile()
    return nc


def _finish(nc):
    return nc


# ---------------------------------------------------------------- launch 2

def build_launch2():
    nc = bacc.Bacc("TRN2", target_bir_lowering=False, debug=False,
                   num_devices=8)

    def inp(name, shape):
        return nc.dram_tensor(name, shape, F32, kind="ExternalInput")

    y_in = nc.dram_tensor("y_in", [Di, T2], BF16, kind="ExternalInput")
    sz_in = nc.dram_tensor("sz_in", [Di, T2], BF16, kind="ExternalInput")
    x_in = inp("x_in", [D, T2])
    ones128 = inp("ones128", [128, 1])
    onorm_g = inp("onorm_g", [Di, 1])
    onorm_b = inp("onorm_b", [Di, 1])
    oproj = inp("oproj", [Di, D])
    fc1p = inp("fc1p", [D, 2 * Di])
    bias1 = inp("bias1", [128, 2])
    fc2w = inp("fc2w", [2 * Di, D])
    fc2b = inp("fc2b", [D, 1])
    out = nc.dram_tensor("out", [D, T2], F32, kind="ExternalOutput")

    with tile.TileContext(nc) as tc, ExitStack() as ctx:
        po = ctx.enter_context(tc.tile_pool(name="main", bufs=1))
        ps = ctx.enter_context(tc.tile_pool(name="psum", bufs=1, space="PSUM"))

        y_sb = po.tile([Di, T2], BF16, tag="y")
        nc.sync.dma_start(y_sb[:], y_in[:])
        sz_sb = po.tile([Di, T2], BF16, tag="sz")
        nc.sync.dma_start(sz_sb[:], sz_in[:])
        x_sb = po.tile([D, T2], F32, tag="x")
        nc.sync.dma_start(x_sb[:], x_in[:])
        ones_sb = po.tile([128, 1], F32, tag="ones")
        nc.sync.dma_start(ones_sb[:], ones128[:])
        ones_bf = po.tile([128, 1], BF16, tag="onesbf")
        nc.vector.memset(ones_bf[:], 1.0)
        og_sb = po.tile([Di, 1], F32, tag="og")
        nc.sync.dma_start(og_sb[:], onorm_g[:])
        ob_sb = po.tile([Di, 1], F32, tag="ob")
        nc.sync.dma_start(ob_sb[:], onorm_b[:])
        op_sb = po.tile([Di, D], F32, tag="oproj")
        nc.sync.dma_start(op_sb[:], oproj[:])
        fc1_sb = po.tile([D, 2 * Di], F32, tag="fc1")
        nc.sync.dma_start(fc1_sb[:], fc1p[:])
        b1_sb = po.tile([128, 2], F32, tag="b1")
        nc.sync.dma_start(b1_sb[:], bias1[:])
        fc2_sb = po.tile([128, 2, D], F32, tag="fc2")
        nc.sync.dma_start(fc2_sb[:], fc2w[:].rearrange("(c p) m -> p c m", p=128))
        fc2b_sb = po.tile([D, 1], F32, tag="fc2b")
        nc.sync.dma_start(fc2b_sb[:], fc2b[:])
        eps_sb = po.tile([128, 1], F32, tag="eps")
        nc.vector.memset(eps_sb[:], EPS)

        def pln(src, parts, tag, dt=F32, ones_t=None):
            """LayerNorm stats over the partition dim of src [parts, T2];
            returns broadcast (mu_b, rs_b) [parts, T2] tiles."""
            ones_t = ones_sb if ones_t is None else ones_t
            sq = po.tile([parts, T2], dt, tag=tag + "sq")
            nc.scalar.square(sq[:], src)
            st0_sb = po.tile([1, T2], F32, tag=tag + "st0sb")
            st1_sb = po.tile([1, T2], F32, tag=tag + "st1sb")
            with tc.tile_pool(name=tag + "ps_st", bufs=1,
                              space="PSUM") as ps_st:
                st0 = ps_st.tile([1, T2], F32, tag="st0")
                st1 = ps_st.tile([1, T2], F32, tag="st1")
                for f in range(T2 // 512):
                    fsl = slice(f * 512, (f + 1) * 512)
                    nc.tensor.matmul(st0[:, fsl], ones_t[0:parts, :],
                                     src[:, fsl], start=True, stop=True)
                    nc.tensor.matmul(st1[:, fsl], ones_t[0:parts, :],
                                     sq[:, fsl], start=True, stop=True)
                nc.scalar.copy(st0_sb[:], st0[:])
                nc.scalar.copy(st1_sb[:], st1[:])
            s0r = po.tile([128, T2 // 128], F32, tag=tag + "s0r")
            s1r = po.tile([128, T2 // 128], F32, tag=tag + "s1r")
            nc.sync.dma_start(s0r[:], st0_sb[:])
            nc.sync.dma_start(s1r[:], st1_sb[:])
            m_r = po.tile([128, T2 // 128], F32, tag=tag + "m")
            nc.scalar.mul(m_r[:], s0r[:], 1.0 / parts)
            msq = po.tile([128, T2 // 128], F32, tag=tag + "msq")
            nc.scalar.square(msq[:], m_r[:])
            v_r = po.tile([128, T2 // 128], F32, tag=tag + "v")
            nc.vector.scalar_tensor_tensor(v_r[:], s1r[:], 1.0 / parts,
                                           msq[:], OP.mult, OP.subtract)
            sd_r = po.tile([128, T2 // 128], F32, tag=tag + "sd")
            nc.scalar.activation(sd_r[:], v_r[:], AF.Sqrt, bias=eps_sb[:parts if False else 128, 0:1])
            rs_r = po.tile([128, T2 // 128], F32, tag=tag + "rs")
            nc.vector.reciprocal(rs_r[:], sd_r[:])
            mu1 = po.tile([1, T2], F32, tag=tag + "mu1")
            rs1 = po.tile([1, T2], F32, tag=tag + "rs1")
            nc.sync.dma_start(mu1[:], m_r[:])
            nc.sync.dma_start(rs1[:], rs_r[:])
            mu_b = po.tile([parts, T2], F32, tag=tag + "mub")
            rs_b = po.tile([parts, T2], F32, tag=tag + "rsb")
            nc.gpsimd.partition_broadcast(mu_b[:], mu1[:])
            nc.gpsimd.partition_broadcast(rs_b[:], rs1[:])
            return mu_b, rs_b

        # out_norm (over Di) + gate
        mu_b, rs_b = pln(y_sb[:], Di, "a", dt=BF16, ones_t=ones_bf)
        t1 = po.tile([Di, T2], F32, tag="t1")
        nc.vector.tensor_tensor(t1[:], y_sb[:], mu_b[:], OP.subtract)
        t2 = po.tile([Di, T2], F32, tag="t2")
        nc.vector.tensor_tensor(t2[:], t1[:], rs_b[:], OP.mult)
        t3 = po.tile([Di, T2], F32, tag="t3")
        nc.vector.tensor_scalar(t3[:], t2[:], og_sb[:, 0:1], ob_sb[:, 0:1],
                                OP.mult, OP.add)
        yg = po.tile([Di, T2], F32, tag="yg")
        nc.vector.tensor_tensor(yg[:], t3[:], sz_sb[:], OP.mult)

        # out_proj + residual ;  "mm" psum tag shared/serialized
        x2 = po.tile([D, T2], F32, tag="x2")
        opps = ps.tile([128, T2], F32, tag="mm")
        for f in range(T2 // 512):
            fsl = slice(f * 512, (f + 1) * 512)
            nc.tensor.matmul(opps[0:D, fsl], op_sb[:], yg[:, fsl],
                             start=True, stop=True)
        nc.vector.tensor_tensor(x2[:], opps[0:D, :], x_sb[:], OP.add)

        # LN2 (over D) -> fc1 -> gelu -> fc2 -> + residual
        mu2, rs2 = pln(x2[:], D, "b")
        h1 = po.tile([D, T2], F32, tag="h1")
        nc.vector.tensor_tensor(h1[:], x2[:], mu2[:], OP.subtract)
        hn = po.tile([D, T2], F32, tag="hn")
        nc.vector.tensor_tensor(hn[:], h1[:], rs2[:], OP.mult)

        g1 = po.tile([128, 2, T2], F32, tag="g1")
        for mc in range(2):
            fp = ps.tile([128, T2], F32, tag="mm")
            for f in range(T2 // 512):
                fsl = slice(f * 512, (f + 1) * 512)
                nc.tensor.matmul(fp[:, fsl],
                                 fc1_sb[:, mc * 128:(mc + 1) * 128],
                                 hn[:, fsl], start=True, stop=True)
            nc.scalar.activation(g1[:, mc, :], fp[:],
                                 AF.Gelu_apprx_tanh, bias=b1_sb[:, mc:mc + 1])
        f2 = ps.tile([128, T2], F32, tag="mm")
        for f in range(T2 // 512):
            fsl = slice(f * 512, (f + 1) * 512)
            for mc in range(2):
                nc.tensor.matmul(f2[0:D, fsl], fc2_sb[:, mc, :],
                                 g1[:, mc, fsl],
                                 start=(mc == 0), stop=(mc == 1))
        o_sb = po.tile([D, T2], F32, tag="o")
        nc.vector.scalar_tensor_tensor(o_sb[:], f2[0:D, :], fc2b_sb[:, 0:1],
                                       x2[:], OP.add, OP.add)
        nc.sync.dma_start(out[:], o_sb[:])

    nc.compile()
    return nc


# ---------------------------------------------------------------- host side

_CACHE = {}


def _get_programs():
    if "nc1" not in _CACHE:
        _CACHE["nc1"] = build_launch1()
        _CACHE["nc2"] = build_launch2()
    return _CACHE["nc1"], _CACHE["nc2"]


def _prep_inmaps(inputs):
    f32 = lambda a: np.ascontiguousarray(np.asarray(a), dtype=np.float32)
    conv_w = f32(inputs["conv_w"])
    conv_b = f32(inputs["conv_b"])
    ln1_g, ln1_b = f32(inputs["ln1_g"]), f32(inputs["ln1_b"])
    in_proj_w = f32(inputs["in_proj_w"])
    dw_w_all = f32(inputs["conv_dw_w"])[:, 0]
    dw_b = f32(inputs["conv_dw_b"])
    x_proj_w = f32(inputs["x_proj_w"])
    dt_proj_w = f32(inputs["dt_proj_w"])
    dt_proj_b = f32(inputs["dt_proj_b"])
    A = np.exp(f32(inputs["A_logs"])).reshape(K, Di, N).astype(np.float32)
    Ds = f32(inputs["Ds"]).reshape(K, Di)

    Wp = (ln1_g[:, None] * in_proj_w).astype(np.float32)        # [64, 256]
    Wp_bf = Wp.astype(ml_dtypes.bfloat16)
    q = Wp.sum(0)
    bias_full = (ln1_b @ in_proj_w).astype(np.float32)          # [256]
    negq = np.ascontiguousarray(np.stack([-q[:Di], -q[Di:]], 1), np.float32)
    sel = np.zeros((128, 2), np.float32)
    sel[:D, 0] = 1.0
    sel[D:, 1] = 1.0
    ones1 = np.ones((1, 128), np.float32)

    Ps = _perms()
    x123 = [np.concatenate([f32(inputs["x1"])[b], f32(inputs["x2"])[b],
                            f32(inputs["x3"])[b]], 0).reshape(3 * C, L)
            for b in range(B)]

    bsel_np = np.zeros((R + 2 * N, N * 128), ml_dtypes.bfloat16)
    csel_np = np.zeros((R + 2 * N, N * 128), ml_dtypes.bfloat16)
    for n in range(N):
        bsel_np[R + n, n * 128:(n + 1) * 128] = 1.0
        csel_np[R + N + n, n * 128:(n + 1) * 128] = 1.0
    shared = {
        "bsel": bsel_np, "csel": csel_np,
        "convT": np.ascontiguousarray(conv_w.T),
        "conv_b": conv_b.reshape(D, 1).copy(),
        "sel": sel, "ones1": ones1, "Wp": Wp_bf, "negq": negq,
        "bias_z": bias_full[Di:].reshape(Di, 1).copy(),
    }
    in_maps = []
    for core in range(8):
        b, k = core // 4, core % 4
        dw_w = _permute_kernel(dw_w_all, k)
        wsum = dw_w.sum((1, 2))
        dwdiag = np.zeros((9, 128, 128), ml_dtypes.bfloat16)
        for t in range(9):
            np.fill_diagonal(dwdiag[t], dw_w[:, t // 3, t % 3])
        in_maps.append({
            **shared,
            "xin": np.ascontiguousarray(x123[b][:, Ps[k]]),
            "dwdiag": dwdiag,
            "bias_dw": (dw_b + bias_full[:Di] * wsum).reshape(Di, 1)
                        .astype(np.float32),
            "xprojT": np.ascontiguousarray(x_proj_w[k].T.astype(ml_dtypes.bfloat16)),
            "dtT": np.ascontiguousarray(dt_proj_w[k].T.astype(ml_dtypes.bfloat16)),
            "dtb": (-dt_proj_b[k]).reshape(Di, 1).copy(),
            "A_in": np.ascontiguousarray(A[k]),
            "Ds_in": Ds[k].reshape(Di, 1).copy(),
        })
    return in_maps, Ps


def _prep_inmaps2(inputs, y_merged, sz_full, x_full):
    f32 = lambda a: np.ascontiguousarray(np.asarray(a), dtype=np.float32)
    ln2_g, ln2_b = f32(inputs["ln2_g"]), f32(inputs["ln2_b"])
    fc1_w, fc1_b = f32(inputs["fc1_w"]), f32(inputs["fc1_b"])
    fc1p = (ln2_g[:, None] * fc1_w).astype(np.float32)
    bias1 = (ln2_b @ fc1_w + fc1_b).astype(np.float32)
    shared = {
        "ones128": np.ones((128, 1), np.float32),
        "onorm_g": f32(inputs["out_norm_g"]).reshape(Di, 1).copy(),
        "onorm_b": f32(inputs["out_norm_b"]).reshape(Di, 1).copy(),
        "oproj": f32(inputs["out_proj_w"]),
        "fc1p": fc1p,
        "bias1": np.ascontiguousarray(np.stack([bias1[:128], bias1[128:]], 1),
                                      np.float32),
        "fc2w": f32(inputs["fc2_w"]),
        "fc2b": f32(inputs["fc2_b"]).reshape(D, 1).copy(),
    }
    in_maps = []
    for core in range(8):
        b, sl = core // 4, slice((core % 4) * T2, (core % 4 + 1) * T2)
        in_maps.append({
            **shared,
            "y_in": np.ascontiguousarray(
                y_merged[b][:, sl].astype(ml_dtypes.bfloat16)),
            "sz_in": np.ascontiguousarray(
                sz_full[b][:, sl].astype(ml_dtypes.bfloat16)),
            "x_in": np.ascontiguousarray(x_full[b][:, sl]),
        })
    return in_maps


def kernel(**inputs):
    nc1, nc2 = _get_programs()
    in_maps, Ps = _prep_inmaps(inputs)
    res1 = run_bass_kernel_spmd(nc1, in_maps, list(range(8))).results

    y_merged = np.zeros((B, Di, L), np.float32)
    sz_full = [None] * B
    x_full = [None] * B
    for core in range(8):
        b, k = core // 4, core % 4
        y_merged[b][:, Ps[k]] += res1[core]["y_out"]
        if k == 0:
            sz_full[b] = res1[core]["sz_out"]
            x_full[b] = res1[core]["x_out"]

    in_maps2 = _prep_inmaps2(inputs, y_merged, sz_full, x_full)
    res2 = run_bass_kernel_spmd(nc2, in_maps2, list(range(8))).results

    out = np.zeros((B, D, L), np.float32)
    for core in range(8):
        b, sl = core // 4, slice((core % 4) * T2, (core % 4 + 1) * T2)
        out[b][:, sl] = res2[core]["out"]
    return out.reshape(B, D, H, W)


# revision 40
# speedup vs baseline: 1.2108x; 1.0018x over previous
"""Trainium2 Bass kernel for nn_Decoder_17489106830107 (VMamba VSSBlock decoder).

Sharding: one (batch, scan-direction) pair per core (B=2 x K=4 = 8 cores).
The host pre-permutes each core's inputs into that core's scan coordinate
order (transpose / 180-rotation of the image), so all 8 cores run ONE
identical SPMD program for launch 1 (conv1x1 -> LN -> in_proj -> depthwise
conv -> x_proj/dt -> 16 hardware linear-recurrence scans on the DVE).
The host then scatter-adds the 4 directional outputs per batch and an
8-way token-parallel launch 2 does the merge epilogue (out_norm, gating,
out_proj, MLP).
"""
import numpy as np
from contextlib import ExitStack

import concourse.bacc as bacc
import concourse.bass as bass
import concourse.mybir as mybir
import concourse.tile as tile
from concourse.bass_utils import run_bass_kernel_spmd
import ml_dtypes

F32 = mybir.dt.float32
F32R = mybir.dt.float32r
BF16 = mybir.dt.bfloat16
AF = mybir.ActivationFunctionType
OP = mybir.AluOpType

B, C, H, W = 2, 256, 64, 64
D = 64
Di = 128
N = 16
R = 4
K = 4
L = H * W          # 4096
LC = 1024          # scan chunk
NCH = L // LC
EPS = 1e-5
T2 = 1024          # launch-2 token slice per core


# ---------------------------------------------------------------- host prep

def _perms():
    ar = np.arange(L)
    p1 = (ar % 64) * 64 + ar // 64
    return [ar, p1, ar[::-1].copy(), p1[::-1].copy()]


def _permute_kernel(w, k):
    if k == 0:
        return w
    if k == 1:
        return w.transpose(0, 2, 1)
    if k == 2:
        return w[:, ::-1, ::-1]
    return w.transpose(0, 2, 1)[:, ::-1, ::-1]


# ---------------------------------------------------------------- launch 1

def _r(ap):
    return ap.bitcast(F32R)


def build_launch1():
    nc = bacc.Bacc("TRN2", target_bir_lowering=False, debug=False,
                   num_devices=8)

    def inp(name, shape):
        return nc.dram_tensor(name, shape, F32, kind="ExternalInput")

    xin = inp("xin", [3 * C, L])
    convT = inp("convT", [3 * C, D])
    conv_b = inp("conv_b", [D, 1])
    sel = inp("sel", [128, 2])
    ones1 = inp("ones1", [1, 128])
    Wp = nc.dram_tensor("Wp", [D, 2 * Di], BF16,
                        kind="ExternalInput")
    negq = inp("negq", [128, 2])
    bias_z = inp("bias_z", [128, 1])
    dwdiag = nc.dram_tensor("dwdiag", [9, 128, 128], BF16,
                            kind="ExternalInput")
    bias_dw = inp("bias_dw", [128, 1])
    xprojT = nc.dram_tensor("xprojT", [Di, R + 2 * N], BF16,
                            kind="ExternalInput")
    dtT = nc.dram_tensor("dtT", [R, Di], BF16,
                         kind="ExternalInput")
    dtb = inp("dtb", [Di, 1])
    A_in = inp("A_in", [Di, N])
    bsel = nc.dram_tensor("bsel", [R + 2 * N, N * 128], BF16,
                          kind="ExternalInput")
    csel = nc.dram_tensor("csel", [R + 2 * N, N * 128], BF16,
                          kind="ExternalInput")
    Ds_in = inp("Ds_in", [Di, 1])

    y_out = nc.dram_tensor("y_out", [Di, L], F32, kind="ExternalOutput")
    sz_out = nc.dram_tensor("sz_out", [Di, L], F32, kind="ExternalOutput")
    x_out = nc.dram_tensor("x_out", [D, L], F32, kind="ExternalOutput")

    with tile.TileContext(nc) as tc, ExitStack() as ctx:
        cpool = ctx.enter_context(tc.tile_pool(name="consts", bufs=1))
        main = ctx.enter_context(tc.tile_pool(name="main", bufs=1))

        # ---- const loads
        convT_sb = cpool.tile([128, 6, D], F32, tag="convT")
        nc.sync.dma_start(convT_sb[:], convT[:].rearrange("(c p) m -> p c m", p=128))
        conv_b_sb = cpool.tile([D, 1], F32, tag="convb")
        nc.sync.dma_start(conv_b_sb[:], conv_b[:])
        sel_sb = cpool.tile([128, 2], F32, tag="sel")
        nc.sync.dma_start(sel_sb[:], sel[:])
        ones1_sb = cpool.tile([1, 128], F32, tag="ones1")
        nc.sync.dma_start(ones1_sb[:], ones1[:])
        Wp_sb = cpool.tile([D, 2 * Di], BF16, tag="Wp")
        nc.sync.dma_start(Wp_sb[:], Wp[:])
        negq_sb = cpool.tile([128, 2], F32, tag="negq")
        nc.sync.dma_start(negq_sb[:], negq[:])
        bias_z_sb = cpool.tile([128, 1], F32, tag="biasz")
        nc.sync.dma_start(bias_z_sb[:], bias_z[:])
        dwdiag_sb = cpool.tile([128, 9, 128], BF16, tag="dwdiag")
        nc.sync.dma_start(dwdiag_sb[:], dwdiag[:].rearrange("t p f -> p t f"))
        bias_dw_sb = cpool.tile([128, 1], F32, tag="biasdw")
        nc.sync.dma_start(bias_dw_sb[:], bias_dw[:])
        xprojT_sb = cpool.tile([Di, R + 2 * N], BF16, tag="xprojT")
        nc.sync.dma_start(xprojT_sb[:], xprojT[:])
        dtT_sb = cpool.tile([R, Di], BF16, tag="dtT")
        nc.sync.dma_start(dtT_sb[:], dtT[:])
        dtb_sb = cpool.tile([Di, 1], F32, tag="dtb")
        nc.sync.dma_start(dtb_sb[:], dtb[:])
        A_sb = cpool.tile([Di, N], F32, tag="A")
        nc.sync.dma_start(A_sb[:], A_in[:])
        Ds_sb = cpool.tile([Di, 1], F32, tag="Ds")
        nc.sync.dma_start(Ds_sb[:], Ds_in[:])
        eps_sb = cpool.tile([128, 1], F32, tag="eps")
        nc.vector.memset(eps_sb[:], EPS)
        bsel_sb = cpool.tile([R + 2 * N, N * 128], BF16, tag="bsel")
        nc.sync.dma_start(bsel_sb[:], bsel[:])
        csel_sb = cpool.tile([R + 2 * N, N * 128], BF16, tag="csel")
        nc.sync.dma_start(csel_sb[:], csel[:])

        # ---- persistent activations
        xc_sb = main.tile([Di, L], BF16, tag="xc")
        xdbl_bf = main.tile([R + 2 * N, L], BF16, tag="xdblbf")
        delta_sb = main.tile([Di, L], F32, tag="delta")
        du_sb = main.tile([Di, L], F32, tag="du")
        carry_sb = main.tile([Di, N], F32, tag="carry")

        with tc.tile_pool(name="imgp", bufs=1) as imgp:
            img = imgp.tile([Di, 66 * 66], BF16, tag="img")
            nc.gpsimd.memset(img[:], 0.0)
            img3 = img[:].rearrange("p (h w) -> p h w", h=66)

            with tc.tile_pool(name="p1", bufs=1) as p1, \
                 tc.tile_pool(name="p1x", bufs=3) as p1x:
                lnin = p1.tile([128, L], F32, tag="lnin")

                # conv1x1: psum[f] accumulates over 6 channel chunks
                with tc.tile_pool(name="ps_conv", bufs=1,
                                  space="PSUM") as ps_conv:
                    cps = [ps_conv.tile([D, 512], F32, tag=f"cps{f}",
                                        name=f"cps{f}")
                           for f in range(8)]
                    for c in range(6):
                        xin_c = p1x.tile([128, L], F32, tag="xin")
                        nc.sync.dma_start(xin_c[:],
                                          xin[:][c * 128:(c + 1) * 128, :])
                        for f in range(8):
                            nc.tensor.matmul(cps[f][:], convT_sb[:, c, :],
                                             xin_c[:, f * 512:(f + 1) * 512],
                                             start=(c == 0), stop=(c == 5))
                    for f in range(8):
                        nc.scalar.activation(lnin[0:D, f * 512:(f + 1) * 512],
                                             cps[f][:], AF.Identity,
                                             bias=conv_b_sb[:, 0:1])
                nc.sync.dma_start(x_out[:], lnin[0:D, :])
                lnin_bf = p1.tile([D, L], BF16, tag="lninbf")
                nc.scalar.copy(lnin_bf[:], lnin[0:D, :])

                # ---- LN1 stats, fully per-half so downstream starts early
                mu_b = p1.tile([128, L], F32, tag="mu_b")
                rs_b = p1.tile([128, L], F32, tag="rs_b")
                with tc.tile_pool(name="ps_st", bufs=1, space="PSUM") as ps_st:
                    for hh in range(2):
                        hsl = slice(hh * 2048, (hh + 1) * 2048)
                        nc.scalar.square(lnin[D:2 * D, hsl], lnin[0:D, hsl])
                        st0 = ps_st.tile([1, L // 2], F32, tag="st0",
                                         name="st0")
                        st1 = ps_st.tile([1, L // 2], F32, tag="st1",
                                         name="st1")
                        for f in range(4):
                            fsl = slice(hh * 2048 + f * 512,
                                        hh * 2048 + (f + 1) * 512)
                            psl = slice(f * 512, (f + 1) * 512)
                            nc.tensor.matmul(st0[:, psl], sel_sb[:, 0:1],
                                             lnin[:, fsl], start=True,
                                             stop=True)
                            nc.tensor.matmul(st1[:, psl], sel_sb[:, 1:2],
                                             lnin[:, fsl], start=True,
                                             stop=True)
                        nc.scalar.copy(mu_b[0:1, hsl], st0[:])
                        nc.scalar.copy(rs_b[0:1, hsl], st1[:])
                        s0r = p1.tile([128, 16], F32, tag="s0r", bufs=2)
                        s1r = p1.tile([128, 16], F32, tag="s1r", bufs=2)
                        nc.sync.dma_start(s0r[:], mu_b[0:1, hsl])
                        nc.sync.dma_start(s1r[:], rs_b[0:1, hsl])
                        m_r = p1.tile([128, 16], F32, tag="m_r", bufs=2)
                        nc.scalar.mul(m_r[:], s0r[:], 1.0 / D)
                        msq = p1.tile([128, 16], F32, tag="msq", bufs=2)
                        nc.scalar.square(msq[:], m_r[:])
                        v_r = p1.tile([128, 16], F32, tag="v_r", bufs=2)
                        nc.vector.scalar_tensor_tensor(v_r[:], s1r[:], 1.0 / D,
                                                       msq[:], OP.mult,
                                                       OP.subtract)
                        sd_r = p1.tile([128, 16], F32, tag="sd_r", bufs=2)
                        nc.scalar.activation(sd_r[:], v_r[:], AF.Sqrt,
                                             bias=eps_sb[:, 0:1])
                        rs_r = p1.tile([128, 16], F32, tag="rs_r", bufs=2)
                        nc.vector.reciprocal(rs_r[:], sd_r[:])
                        nc.sync.dma_start(mu_b[0:1, hsl], m_r[:])
                        nc.sync.dma_start(rs_b[0:1, hsl], rs_r[:])
                        nc.gpsimd.partition_broadcast(mu_b[:, hsl],
                                                      mu_b[0:1, hsl])
                        nc.gpsimd.partition_broadcast(rs_b[:, hsl],
                                                      rs_b[0:1, hsl])

                # ---- in_proj + LN fixup
                with tc.tile_pool(name="ps_ip", bufs=4, space="PSUM") as ps_ip, \
                     tc.tile_pool(name="fix", bufs=3) as fix:
                    for mc in range(2):
                        for f in range(8):
                            fsl = slice(f * 512, (f + 1) * 512)
                            pp = ps_ip.tile([128, 512], F32, tag="ipps")
                            nc.tensor.matmul(pp[:],
                                             Wp_sb[:, mc * 128:(mc + 1) * 128],
                                             lnin_bf[:, fsl],
                                             start=True, stop=True)
                            t1 = fix.tile([128, 512], F32, tag="t1")
                            nc.vector.scalar_tensor_tensor(
                                t1[:], mu_b[:, fsl], negq_sb[:, mc:mc + 1],
                                pp[:], OP.mult, OP.add)
                            if mc == 0:
                                r0 = f * 8
                                nc.vector.tensor_tensor(
                                    img3[:, 1 + r0:1 + r0 + 8, 1:65],
                                    t1[:].rearrange("p (r w) -> p r w", r=8),
                                    rs_b[:, fsl].rearrange("p (r w) -> p r w",
                                                           r=8),
                                    OP.mult)
                            else:
                                t2 = fix.tile([128, 512], F32, tag="t2")
                                nc.vector.tensor_tensor(t2[:], t1[:],
                                                        rs_b[:, fsl], OP.mult)
                                szt = fix.tile([128, 512], F32, tag="szt")
                                nc.scalar.activation(szt[:], t2[:],
                                                     AF.Silu,
                                                     bias=bias_z_sb[:, 0:1])
                                nc.sync.dma_start(sz_out[:][:, fsl], szt[:])

            # ---- interleaved per-chunk pipeline: dwconv -> x_proj ->
            # delta -> scans, so chunk c's scans overlap chunk c+1's prep
            dts_sb = main.tile([R, L], BF16, tag="dts")
            with tc.tile_pool(name="ps_dw", bufs=1, space="PSUM") as ps_dw, \
                 tc.tile_pool(name="ps_xp", bufs=1, space="PSUM") as ps_xp, \
                 tc.tile_pool(name="ps_dt", bufs=1, space="PSUM") as ps_dt, \
                 tc.tile_pool(name="ps_bb", bufs=1, space="PSUM") as ps_bb, \
                 tc.tile_pool(name="nl", bufs=4) as nl, \
                 tc.tile_pool(name="yp", bufs=2) as yp:
            
                for c in range(NCH):
                    csl = slice(c * LC, (c + 1) * LC)
                    for f in range(2 * c, 2 * c + 2):
                        fsl = slice(f * 512, (f + 1) * 512)
                        r0 = f * 8
                        dps = ps_dw.tile([128, 512], F32, tag="dwps")
                        for t in range(9):
                            di_, dj = t // 3, t % 3
                            nc.tensor.matmul(
                                dps[:], dwdiag_sb[:, t, :],
                                img3[:, r0 + di_:r0 + di_ + 8, dj:dj + 64],
                                start=(t == 0), stop=(t == 8))
                        nc.scalar.activation(xc_sb[:, fsl], dps[:], AF.Silu,
                                             bias=bias_dw_sb[:, 0:1])
                        xps = ps_xp.tile([R + 2 * N, 512], F32, tag="xpps")
                        nc.tensor.matmul(xps[:], xprojT_sb[:], xc_sb[:, fsl],
                                         start=True, stop=True)
                        nc.scalar.copy(xdbl_bf[:, fsl], xps[:])
                        nc.scalar.copy(dts_sb[:, fsl], xps[0:R, :])
                        dtps = ps_dt.tile([Di, 512], F32, tag="dtps")
                        nc.tensor.matmul(dtps[:], dtT_sb[:], dts_sb[:, fsl],
                                         start=True, stop=True)
                        nc.scalar.activation(delta_sb[:, fsl], dtps[:],
                                             AF.Sigmoid,
                                             bias=dtb_sb[:, 0:1], scale=-1.0)
                    nc.scalar.activation(delta_sb[:, csl], delta_sb[:, csl],
                                         AF.Ln)
                    nc.vector.scalar_tensor_tensor(du_sb[:, csl],
                                                   delta_sb[:, csl],
                                                   -1.0, xc_sb[:, csl],
                                                   OP.mult, OP.mult)
                    y_acc = yp.tile([Di, LC], F32, tag="yacc")
                    for n in range(N):
                        bb = ps_bb.tile([128, LC], F32, tag="bb")
                        for j in range(LC // 512):
                            nc.tensor.matmul(
                                bb[:, j * 512:(j + 1) * 512],
                                bsel_sb[:, n * 128:(n + 1) * 128],
                                xdbl_bf[:, c * LC + j * 512:
                                        c * LC + (j + 1) * 512],
                                start=True, stop=True)
                        cb = ps_bb.tile([128, LC], F32, tag="cb")
                        for j in range(LC // 512):
                            nc.tensor.matmul(
                                cb[:, j * 512:(j + 1) * 512],
                                csel_sb[:, n * 128:(n + 1) * 128],
                                xdbl_bf[:, c * LC + j * 512:
                                        c * LC + (j + 1) * 512],
                                start=True, stop=True)
                        da = nl.tile([Di, LC], F32, tag="da")
                        nc.scalar.activation(da[:], delta_sb[:, csl], AF.Exp,
                                             scale=A_sb[:, n:n + 1])
                        dbu = nl.tile([Di, LC], F32, tag="dbu")
                        nc.vector.tensor_tensor(dbu[:], du_sb[:, csl], bb[:],
                                                OP.mult)
                        h = nl.tile([Di, LC], F32, tag="h")
                        nc.vector.tensor_tensor_scan(
                            h[:], da[:], dbu[:],
                            0.0 if c == 0 else carry_sb[:, n:n + 1],
                            OP.mult, OP.add)
                        if c < NCH - 1:
                            nc.scalar.copy(carry_sb[:, n:n + 1],
                                           h[:, LC - 1:LC])
                        if n == 0:
                            nc.vector.tensor_tensor(y_acc[:], h[:], cb[:],
                                                    OP.mult)
                        else:
                            tmp = nl.tile([Di, LC], F32, tag="tmp")
                            nc.vector.tensor_tensor(tmp[:], h[:], cb[:],
                                                    OP.mult)
                            nc.gpsimd.tensor_tensor(y_acc[:], y_acc[:],
                                                    tmp[:], OP.add)
                    y_f = yp.tile([Di, LC], F32, tag="yout")
                    nc.vector.scalar_tensor_tensor(y_f[:], xc_sb[:, csl],
                                                   Ds_sb[:, 0:1], y_acc[:],
                                                   OP.mult, OP.add)
                    nc.sync.dma_start(y_out[:][:, csl], y_f[:])

    nc.compile()
    return nc


def _finish(nc):
    return nc


# ---------------------------------------------------------------- launch 2

def build_launch2():
    nc = bacc.Bacc("TRN2", target_bir_lowering=False, debug=False,
                   num_devices=8)

    def inp(name, shape):
        return nc.dram_tensor(name, shape, F32, kind="ExternalInput")

    y_in = nc.dram_tensor("y_in", [Di, T2], BF16, kind="ExternalInput")
    sz_in = nc.dram_tensor("sz_in", [Di, T2], BF16, kind="ExternalInput")
    x_in = inp("x_in", [D, T2])
    ones128 = inp("ones128", [128, 1])
    onorm_g = inp("onorm_g", [Di, 1])
    onorm_b = inp("onorm_b", [Di, 1])
    oproj = inp("oproj", [Di, D])
    fc1p = inp("fc1p", [D, 2 * Di])
    bias1 = inp("bias1", [128, 2])
    fc2w = inp("fc2w", [2 * Di, D])
    fc2b = inp("fc2b", [D, 1])
    out = nc.dram_tensor("out", [D, T2], F32, kind="ExternalOutput")

    with tile.TileContext(nc) as tc, ExitStack() as ctx:
        po = ctx.enter_context(tc.tile_pool(name="main", bufs=1))
        ps = ctx.enter_context(tc.tile_pool(name="psum", bufs=1, space="PSUM"))

        y_sb = po.tile([Di, T2], BF16, tag="y")
        nc.sync.dma_start(y_sb[:], y_in[:])
        sz_sb = po.tile([Di, T2], BF16, tag="sz")
        nc.sync.dma_start(sz_sb[:], sz_in[:])
        x_sb = po.tile([D, T2], F32, tag="x")
        nc.sync.dma_start(x_sb[:], x_in[:])
        ones_sb = po.tile([128, 1], F32, tag="ones")
        nc.sync.dma_start(ones_sb[:], ones128[:])
        ones_bf = po.tile([128, 1], BF16, tag="onesbf")
        nc.vector.memset(ones_bf[:], 1.0)
        og_sb = po.tile([Di, 1], F32, tag="og")
        nc.sync.dma_start(og_sb[:], onorm_g[:])
        ob_sb = po.tile([Di, 1], F32, tag="ob")
        nc.sync.dma_start(ob_sb[:], onorm_b[:])
        op_sb = po.tile([Di, D], F32, tag="oproj")
        nc.sync.dma_start(op_sb[:], oproj[:])
        fc1_sb = po.tile([D, 2 * Di], F32, tag="fc1")
        nc.sync.dma_start(fc1_sb[:], fc1p[:])
        b1_sb = po.tile([128, 2], F32, tag="b1")
        nc.sync.dma_start(b1_sb[:], bias1[:])
        fc2_sb = po.tile([128, 2, D], F32, tag="fc2")
        nc.sync.dma_start(fc2_sb[:], fc2w[:].rearrange("(c p) m -> p c m", p=128))
        fc2b_sb = po.tile([D, 1], F32, tag="fc2b")
        nc.sync.dma_start(fc2b_sb[:], fc2b[:])
        eps_sb = po.tile([128, 1], F32, tag="eps")
        nc.vector.memset(eps_sb[:], EPS)

        def pln(src, parts, tag, dt=F32, ones_t=None):
            """LayerNorm stats over the partition dim of src [parts, T2];
            returns broadcast (mu_b, rs_b) [parts, T2] tiles."""
            ones_t = ones_sb if ones_t is None else ones_t
            sq = po.tile([parts, T2], dt, tag=tag + "sq")
            nc.scalar.square(sq[:], src)
            st0_sb = po.tile([1, T2], F32, tag=tag + "st0sb")
            st1_sb = po.tile([1, T2], F32, tag=tag + "st1sb")
            with tc.tile_pool(name=tag + "ps_st", bufs=1,
                              space="PSUM") as ps_st:
                st0 = ps_st.tile([1, T2], F32, tag="st0")
                st1 = ps_st.tile([1, T2], F32, tag="st1")
                for f in range(T2 // 512):
                    fsl = slice(f * 512, (f + 1) * 512)
                    nc.tensor.matmul(st0[:, fsl], ones_t[0:parts, :],
                                     src[:, fsl], start=True, stop=True)
                    nc.tensor.matmul(st1[:, fsl], ones_t[0:parts, :],
                                     sq[:, fsl], start=True, stop=True)
                nc.scalar.copy(st0_sb[:], st0[:])
                nc.scalar.copy(st1_sb[:], st1[:])
            s0r = po.tile([128, T2 // 128], F32, tag=tag + "s0r")
            s1r = po.tile([128, T2 // 128], F32, tag=tag + "s1r")
            nc.sync.dma_start(s0r[:], st0_sb[:])
            nc.sync.dma_start(s1r[:], st1_sb[:])
            m_r = po.tile([128, T2 // 128], F32, tag=tag + "m")
            nc.scalar.mul(m_r[:], s0r[:], 1.0 / parts)
            msq = po.tile([128, T2 // 128], F32, tag=tag + "msq")
            nc.scalar.square(msq[:], m_r[:])
            v_r = po.tile([128, T2 // 128], F32, tag=tag + "v")
            nc.vector.scalar_tensor_tensor(v_r[:], s1r[:], 1.0 / parts,
                                           msq[:], OP.mult, OP.subtract)
            sd_r = po.tile([128, T2 // 128], F32, tag=tag + "sd")
            nc.scalar.activation(sd_r[:], v_r[:], AF.Sqrt, bias=eps_sb[:parts if False else 128, 0:1])
            rs_r = po.tile([128, T2 // 128], F32, tag=tag + "rs")
            nc.vector.reciprocal(rs_r[:], sd_r[:])
            mu1 = po.tile([1, T2], F32, tag=tag + "mu1")
            rs1 = po.tile([1, T2], F32, tag=tag + "rs1")
            nc.sync.dma_start(mu1[:], m_r[:])
            nc.sync.dma_start(rs1[:], rs_r[:])
            mu_b = po.tile([parts, T2], F32, tag=tag + "mub")
            rs_b = po.tile([parts, T2], F32, tag=tag + "rsb")
            nc.gpsimd.partition_broadcast(mu_b[:], mu1[:])
            nc.gpsimd.partition_broadcast(rs_b[:], rs1[:])
            return mu_b, rs_b

        # out_norm (over Di) + gate
        mu_b, rs_b = pln(y_sb[:], Di, "a", dt=BF16, ones_t=ones_bf)
        t1 = po.tile([Di, T2], F32, tag="t1")
        nc.vector.tensor_tensor(t1[:], y_sb[:], mu_b[:], OP.subtract)
        t2 = po.tile([Di, T2], F32, tag="t2")
        nc.vector.tensor_tensor(t2[:], t1[:], rs_b[:], OP.mult)
        t3 = po.tile([Di, T2], F32, tag="t3")
        nc.vector.tensor_scalar(t3[:], t2[:], og_sb[:, 0:1], ob_sb[:, 0:1],
                                OP.mult, OP.add)
        yg = po.tile([Di, T2], F32, tag="yg")
        nc.vector.tensor_tensor(yg[:], t3[:], sz_sb[:], OP.mult)

        # out_proj + residual ;  "mm" psum tag shared/serialized
        x2 = po.tile([D, T2], F32, tag="x2")
        opps = ps.tile([128, T2], F32, tag="mm")
        for f in range(T2 // 512):
            fsl = slice(f * 512, (f + 1) * 512)
            nc.tensor.matmul(opps[0:D, fsl], op_sb[:], yg[:, fsl],
                             start=True, stop=True)
        nc.vector.tensor_tensor(x2[:], opps[0:D, :], x_sb[:], OP.add)

        # LN2 (over D) -> fc1 -> gelu -> fc2 -> + residual
        mu2, rs2 = pln(x2[:], D, "b")
        h1 = po.tile([D, T2], F32, tag="h1")
        nc.vector.tensor_tensor(h1[:], x2[:], mu2[:], OP.subtract)
        hn = po.tile([D, T2], F32, tag="hn")
        nc.vector.tensor_tensor(hn[:], h1[:], rs2[:], OP.mult)

        g1 = po.tile([128, 2, T2], F32, tag="g1")
        for mc in range(2):
            fp = ps.tile([128, T2], F32, tag="mm")
            for f in range(T2 // 512):
                fsl = slice(f * 512, (f + 1) * 512)
                nc.tensor.matmul(fp[:, fsl],
                                 fc1_sb[:, mc * 128:(mc + 1) * 128],
                                 hn[:, fsl], start=True, stop=True)
            nc.scalar.activation(g1[:, mc, :], fp[:],
                                 AF.Gelu_apprx_tanh, bias=b1_sb[:, mc:mc + 1])
        f2 = ps.tile([128, T2], F32, tag="mm")
        for f in range(T2 // 512):
            fsl = slice(f * 512, (f + 1) * 512)
            for mc in range(2):
                nc.tensor.matmul(f2[0:D, fsl], fc2_sb[:, mc, :],
                                 g1[:, mc, fsl],
                                 start=(mc == 0), stop=(mc == 1))
        o_sb = po.tile([D, T2], F32, tag="o")
        nc.vector.scalar_tensor_tensor(o_sb[:], f2[0:D, :], fc2b_sb[:, 0:1],
                                       x2[:], OP.add, OP.add)
        nc.sync.dma_start(out[:], o_sb[:])

    nc.compile()
    return nc


# ---------------------------------------------------------------- host side

_CACHE = {}


def _get_programs():
    if "nc1" not in _CACHE:
        _CACHE["nc1"] = build_launch1()
        _CACHE["nc2"] = build_launch2()
    return _CACHE["nc1"], _CACHE["nc2"]


def _prep_inmaps(inputs):
    f32 = lambda a: np.ascontiguousarray(np.asarray(a), dtype=np.float32)
    conv_w = f32(inputs["conv_w"])
    conv_b = f32(inputs["conv_b"])
    ln1_g, ln1_b = f32(inputs["ln1_g"]), f32(inputs["ln1_b"])
    in_proj_w = f32(inputs["in_proj_w"])
    dw_w_all = f32(inputs["conv_dw_w"])[:, 0]
    dw_b = f32(inputs["conv_dw_b"])
    x_proj_w = f32(inputs["x_proj_w"])
    dt_proj_w = f32(inputs["dt_proj_w"])
    dt_proj_b = f32(inputs["dt_proj_b"])
    A = np.exp(f32(inputs["A_logs"])).reshape(K, Di, N).astype(np.float32)
    Ds = f32(inputs["Ds"]).reshape(K, Di)

    Wp = (ln1_g[:, None] * in_proj_w).astype(np.float32)        # [64, 256]
    Wp_bf = Wp.astype(ml_dtypes.bfloat16)
    q = Wp.sum(0)
    bias_full = (ln1_b @ in_proj_w).astype(np.float32)          # [256]
    negq = np.ascontiguousarray(np.stack([-q[:Di], -q[Di:]], 1), np.float32)
    sel = np.zeros((128, 2), np.float32)
    sel[:D, 0] = 1.0
    sel[D:, 1] = 1.0
    ones1 = np.ones((1, 128), np.float32)

    Ps = _perms()
    x123 = [np.concatenate([f32(inputs["x1"])[b], f32(inputs["x2"])[b],
                            f32(inputs["x3"])[b]], 0).reshape(3 * C, L)
            for b in range(B)]

    bsel_np = np.zeros((R + 2 * N, N * 128), ml_dtypes.bfloat16)
    csel_np = np.zeros((R + 2 * N, N * 128), ml_dtypes.bfloat16)
    for n in range(N):
        bsel_np[R + n, n * 128:(n + 1) * 128] = 1.0
        csel_np[R + N + n, n * 128:(n + 1) * 128] = 1.0
    shared = {
        "bsel": bsel_np, "csel": csel_np,
        "convT": np.ascontiguousarray(conv_w.T),
        "conv_b": conv_b.reshape(D, 1).copy(),
        "sel": sel, "ones1": ones1, "Wp": Wp_bf, "negq": negq,
        "bias_z": bias_full[Di:].reshape(Di, 1).copy(),
    }
    in_maps = []
    for core in range(8):
        b, k = core // 4, core % 4
        dw_w = _permute_kernel(dw_w_all, k)
        wsum = dw_w.sum((1, 2))
        dwdiag = np.zeros((9, 128, 128), ml_dtypes.bfloat16)
        for t in range(9):
            np.fill_diagonal(dwdiag[t], dw_w[:, t // 3, t % 3])
        in_maps.append({
            **shared,
            "xin": np.ascontiguousarray(x123[b][:, Ps[k]]),
            "dwdiag": dwdiag,
            "bias_dw": (dw_b + bias_full[:Di] * wsum).reshape(Di, 1)
                        .astype(np.float32),
            "xprojT": np.ascontiguousarray(x_proj_w[k].T.astype(ml_dtypes.bfloat16)),
            "dtT": np.ascontiguousarray(dt_proj_w[k].T.astype(ml_dtypes.bfloat16)),
            "dtb": (-dt_proj_b[k]).reshape(Di, 1).copy(),
            "A_in": np.ascontiguousarray(A[k]),
            "Ds_in": Ds[k].reshape(Di, 1).copy(),
        })
    return in_maps, Ps


def _prep_inmaps2(inputs, y_merged, sz_full, x_full):
    f32 = lambda a: np.ascontiguousarray(np.asarray(a), dtype=np.float32)
    ln2_g, ln2_b = f32(inputs["ln2_g"]), f32(inputs["ln2_b"])
    fc1_w, fc1_b = f32(inputs["fc1_w"]), f32(inputs["fc1_b"])
    fc1p = (ln2_g[:, None] * fc1_w).astype(np.float32)
    bias1 = (ln2_b @ fc1_w + fc1_b).astype(np.float32)
    shared = {
        "ones128": np.ones((128, 1), np.float32),
        "onorm_g": f32(inputs["out_norm_g"]).reshape(Di, 1).copy(),
        "onorm_b": f32(inputs["out_norm_b"]).reshape(Di, 1).copy(),
        "oproj": f32(inputs["out_proj_w"]),
        "fc1p": fc1p,
        "bias1": np.ascontiguousarray(np.stack([bias1[:128], bias1[128:]], 1),
                                      np.float32),
        "fc2w": f32(inputs["fc2_w"]),
        "fc2b": f32(inputs["fc2_b"]).reshape(D, 1).copy(),
    }
    in_maps = []
    for core in range(8):
        b, sl = core // 4, slice((core % 4) * T2, (core % 4 + 1) * T2)
        in_maps.append({
            **shared,
            "y_in": np.ascontiguousarray(
                y_merged[b][:, sl].astype(ml_dtypes.bfloat16)),
            "sz_in": np.ascontiguousarray(
                sz_full[b][:, sl].astype(ml_dtypes.bfloat16)),
            "x_in": np.ascontiguousarray(x_full[b][:, sl]),
        })
    return in_maps


def kernel(**inputs):
    nc1, nc2 = _get_programs()
    in_maps, Ps = _prep_inmaps(inputs)
    res1 = run_bass_kernel_spmd(nc1, in_maps, list(range(8))).results

    y_merged = np.zeros((B, Di, L), np.float32)
    sz_full = [None] * B
    x_full = [None] * B
    for core in range(8):
        b, k = core // 4, core % 4
        y_merged[b][:, Ps[k]] += res1[core]["y_out"]
        if k == 0:
            sz_full[b] = res1[core]["sz_out"]
            x_full[b] = res1[core]["x_out"]

    in_maps2 = _prep_inmaps2(inputs, y_merged, sz_full, x_full)
    res2 = run_bass_kernel_spmd(nc2, in_maps2, list(range(8))).results

    out = np.zeros((B, D, L), np.float32)
    for core in range(8):
        b, sl = core // 4, slice((core % 4) * T2, (core % 4 + 1) * T2)
        out[b][:, sl] = res2[core]["out"]
    return out.reshape(B, D, H, W)
